# revision 24
# baseline (speedup 1.0000x reference)
"""Fused LN + QKV + RoPE + attention + out-proj Trainium2 kernel, v4.

Shapes (hardcoded from the problem spec):
  x [4, 2048, 512] fp32, w_qkv [512, 1536], w_out [512, 512],
  ln_gamma/ln_beta/b_out [512]. 8 heads of 64. Output [4, 2048, 512].

Sharding: 8 cores = 4 batches x 2 head-groups (4 heads each). Each core
computes a w_out row-split partial output for its batch; the host sums
the two partials per batch and adds b_out.

Design notes (ACT-exp is the roofline: 16.8M exp/core ~= 110us min):
 - LN is two-pass: per-tile sum (DVE reduce) and sum-of-squares, then
   ONE batched sqrt + reciprocal, then per-tile xn (bf16 so the PE
   transpose runs at 1 cycle/row).
 - QKV: 6 M-tiles (k/q/v per pair); RoPE's roll computed by a
   block-diagonal permutation matmul on (q + beta); combine split
   across GpSimd (t*cos) and DVE (pr*sin, final bf16 add in 2x mode).
 - Attention per head-pair, per-mt software pipeline: QK row-tiled
   2 heads concurrently in the 128x128 PE array into one [128,2,512]
   PSUM slab (2 rotating slabs), ONE exp per mt covering both heads so
   ACT runs back-to-back; PV (ones-augmented V, M=65, row 64 = softmax
   denominator) trails exp by one mt. PE order QK(mt+1) before PV(mt)
   so the in-order PE never blocks the exp chain.
 - Normalize: reciprocal_approx_fast (~5x faster than DVE RECIPROCAL)
   on both heads' D rows, fp32 ones-matmul broadcast into rows 64:128
   of the acc's own psum banks, one fused scalar_tensor_tensor
   (acc * 1/D) per head. Emitted at the START of the next qchunk.
 - Out-proj interleaved: emitted per-qchunk during pair 1's attention
   (needs both pairs' outn), evacuated on DVE (ACT is exp-saturated).
 - PSUM: s3 slabs 2x2 + acc 2 + out-proj 2 = 8 banks exactly.
Matmul operands bf16 (fp32 PSUM accumulation); LN/softmax math fp32.
"""

import numpy as np

import concourse.bass as bass
import concourse.tile as tile
from concourse import mybir
from concourse.bass_utils import run_bass_kernel_spmd

F32 = mybir.dt.float32
BF16 = mybir.dt.bfloat16
AX = mybir.AxisListType
OP = mybir.AluOpType
ACT = mybir.ActivationFunctionType

B, N, D = 4, 2048, 512
HEADS, DH = 8, 64
HPC = 4            # heads per core
EPS = 1e-5
NT = N // 128      # 16 token tiles
KT = D // 128      # 4 feature tiles
W = 512            # attention query-chunk width


def _split_multiwait(nc):
    """Insert NoOps so no instruction carries more than one sem wait.

    The pinned walrus rejects >1 sync wait per instruction
    (setupSyncWait "Too many sync wait commands"). Waits are a
    conjunction, so hoisting all but the last onto same-engine NoOps
    immediately before the instruction is equivalent.
    """
    ctr = 0
    for fn in nc.m.functions:
        for blk in fn.blocks:
            insts = blk.instructions
            idx = 0
            while idx < len(insts):
                inst = insts[idx]
                si = inst.sync_info
                if si is not None and len(si.on_wait) > 1:
                    waits = list(si.on_wait)
                    for w in waits[:-1]:
                        nop = mybir.InstNoOp(name=f"SWNOP-{ctr}", ins=[], outs=[])
                        ctr += 1
                        nop.engine = inst.engine
                        nop.sync_info = mybir.SyncInfo(on_wait=[w], on_update=[])
                        insts.insert(idx, nop)
                        idx += 1
                    inst.sync_info = mybir.SyncInfo(
                        on_wait=[waits[-1]], on_update=list(si.on_update)
                    )
                idx += 1


def build_nc(loops=1):
    from contextlib import ExitStack

    nc = bass.Bass("TRN2", target_bir_lowering=False, num_devices=8)

    x_nat = nc.dram_tensor("x_nat", [N, D], BF16, kind="ExternalInput")
    # gamma-folded QKV weights bf16, M-tile order k01 q01 v01 k23 q23 v23
    wqkv = nc.dram_tensor("wqkv", [D, 768], BF16, kind="ExternalInput")
    beta_mt = nc.dram_tensor("beta_mt", [128, 6], F32, kind="ExternalInput")
    r2 = nc.dram_tensor("r2", [128, 128], BF16, kind="ExternalInput")
    wout = nc.dram_tensor("wout", [HPC * DH, D], BF16, kind="ExternalInput")
    cos2 = nc.dram_tensor("cos2", [128, N], BF16, kind="ExternalInput")
    sin2 = nc.dram_tensor("sin2", [128, N], BF16, kind="ExternalInput")
    ident = nc.dram_tensor("ident", [128, 128], F32, kind="ExternalInput")
    y = nc.dram_tensor("y", [D, N], BF16, kind="ExternalOutput")

    with tile.TileContext(nc) as tc:
      for _loop in range(loops):
        with ExitStack() as ctx:
          const = ctx.enter_context(tc.tile_pool(name="const", bufs=1))
          qk_pool = ctx.enter_context(tc.tile_pool(name="qk", bufs=1))
          va_pool = ctx.enter_context(tc.tile_pool(name="va", bufs=1))
          outn_pool = ctx.enter_context(tc.tile_pool(name="outn", bufs=1))

          ident_sb = const.tile([128, 128], F32)
          nc.gpsimd.dma_start(ident_sb[:], ident[:, :])
          ident_bf = const.tile([128, 128], BF16)
          nc.vector.tensor_copy(ident_bf[:], ident_sb[:])
          r2_sb = const.tile([128, 128], BF16)
          nc.gpsimd.dma_start(r2_sb[:], r2[:, :])
          eps_sb = const.tile([128, 1], F32)
          nc.vector.memset(eps_sb[:], EPS)
          ones_f32 = const.tile([1, 64], F32)
          nc.vector.memset(ones_f32[:], 1.0)
          ones_bf = const.tile([1, 64], BF16)
          nc.vector.memset(ones_bf[:], 1.0)
          beta_sb = const.tile([128, 6], F32)
          nc.gpsimd.dma_start(beta_sb[:], beta_mt[:, :])
          wq_sb = const.tile([128, KT, 768], BF16, name="wq")
          for kt in range(KT):
              nc.gpsimd.dma_start(wq_sb[:, kt, :],
                                  wqkv[kt * 128:(kt + 1) * 128, :])
          wout_sb = const.tile([128, 2, D], BF16, name="wout")
          for p in range(2):
              nc.scalar.dma_start(wout_sb[:, p, :],
                                  wout[p * 128:(p + 1) * 128, :])
          cos_sb = const.tile([128, N], BF16, name="cos")
          nc.scalar.dma_start(cos_sb[:], cos2[:, :])
          sin_sb = const.tile([128, N], BF16, name="sin")
          nc.scalar.dma_start(sin_sb[:], sin2[:, :])

          # q/k rope'd feature-major per pair [128, N]; vap per pair holds
          # both heads' V ktok-major with ones columns at 64 and 129.
          qs = [qk_pool.tile([128, N], BF16, name=f"qs{p}", tag=f"qs{p}")
                for p in range(2)]
          ks = [qk_pool.tile([128, N], BF16, name=f"ks{p}", tag=f"ks{p}")
                for p in range(2)]
          vap = [va_pool.tile([128, NT, 130], BF16, name=f"vap{p}",
                              tag=f"vap{p}") for p in range(2)]
          for p in range(2):
              nc.vector.memset(vap[p][:], 1.0)
          outn = [outn_pool.tile([128, N], BF16, name=f"on{p}", tag=f"on{p}")
                  for p in range(2)]

          # ---- Stage A: LayerNorm (two-pass) + PE transpose ----
          with ExitStack() as s1:
              x_p = s1.enter_context(tc.tile_pool(name="x", bufs=1))
              st_p = s1.enter_context(tc.tile_pool(name="st", bufs=1))
              xn_p = s1.enter_context(tc.tile_pool(name="xn", bufs=3))
              scr_p = s1.enter_context(tc.tile_pool(name="scr", bufs=2))
              xnT_p = s1.enter_context(tc.tile_pool(name="xnT", bufs=1))
              ptA_ps = s1.enter_context(tc.tile_pool(name="ptA", bufs=1,
                                                     space="PSUM"))

              xts = x_p.tile([128, NT, D], BF16, name="xts")
              muvar = st_p.tile([128, NT, 2], F32, name="muvar")
              for tt in range(NT):
                  nc.sync.dma_start(xts[:, tt, :],
                                    x_nat[tt * 128:(tt + 1) * 128, :])
                  bn6 = scr_p.tile([128, 6], F32, tag="bn6")
                  nc.vector.bn_stats(bn6[:], xts[:, tt, :])
                  nc.vector.bn_aggr(muvar[:, tt, :], bn6[:])
              mu_all = muvar[:, :, 0:1].rearrange("p a b -> p (a b)")
              sd_all = st_p.tile([128, NT], F32, name="sd_all")
              nc.scalar.activation(sd_all[:],
                                   muvar[:, :, 1:2].rearrange(
                                       "p a b -> p (a b)"),
                                   ACT.Sqrt, bias=eps_sb[:])
              rs_all = st_p.tile([128, NT], F32, name="rs_all")
              nc.vector.reciprocal(rs_all[:], sd_all[:])
              bias2 = st_p.tile([128, NT], F32, name="bias2")
              nc.vector.scalar_tensor_tensor(
                  bias2[:], mu_all[:], -1.0, rs_all[:], op0=OP.mult,
                  op1=OP.mult)

              xnT = xnT_p.tile([128, KT, N], BF16, name="xnT")
              for tt in range(NT):
                  xn = xn_p.tile([128, D], BF16, tag="xn")
                  if tt % 2 == 0:
                      nc.scalar.activation(xn[:], xts[:, tt, :], ACT.Identity,
                                           bias=bias2[:, tt:tt + 1],
                                           scale=rs_all[:, tt:tt + 1])
                  else:
                      nc.vector.tensor_scalar(
                          xn[:], xts[:, tt, :], muvar[:, tt, 0:1],
                          rs_all[:, tt:tt + 1], op0=OP.subtract, op1=OP.mult)
                  pt = ptA_ps.tile([128, KT, 128], BF16, tag="pt")
                  for ft in range(KT):
                      nc.tensor.transpose(
                          pt[:, ft, :], xn[:, ft * 128:(ft + 1) * 128],
                          ident_bf[:])
                  nc.scalar.copy(
                      xnT[:, :, tt * 128:(tt + 1) * 128], pt[:])

              # ---- Stage B: QKV + RoPE per pair ----
              with ExitStack() as s2:
                  pq_ps = s2.enter_context(tc.tile_pool(name="pq", bufs=2,
                                                        space="PSUM"))
                  pr_ps = s2.enter_context(tc.tile_pool(name="pr", bufs=1,
                                                        space="PSUM"))
                  ptV_ps = s2.enter_context(tc.tile_pool(name="ptV", bufs=1,
                                                         space="PSUM"))
                  t_p = s2.enter_context(tc.tile_pool(name="t", bufs=3))
                  t1_p = s2.enter_context(tc.tile_pool(name="t1", bufs=2))
                  vsb_p = s2.enter_context(tc.tile_pool(name="vsb", bufs=2))

                  def bm(m):
                      return beta_sb[:, m:m + 1]

                  def qkv_mm(psum_ap, m, half):
                      ms = slice(m * 128, (m + 1) * 128)
                      for nn in range(2):
                          cs = slice(half * 1024 + nn * 512,
                                     half * 1024 + (nn + 1) * 512)
                          for kt in range(KT):
                              nc.tensor.matmul(
                                  psum_ap[:, nn * 512:(nn + 1) * 512],
                                  wq_sb[:, kt, ms], xnT[:, kt, cs],
                                  start=(kt == 0), stop=(kt == KT - 1))

                  for p in range(2):
                      vsb = vsb_p.tile([128, N], BF16, tag=f"vsb{p}")
                      for half in range(2):
                          hs = slice(half * 1024, (half + 1) * 1024)
                          for sec, dst in ((0, ks[p]), (1, qs[p])):
                              m = 3 * p + sec
                              pq = pq_ps.tile([128, 1024], F32, tag="pq")
                              qkv_mm(pq, m, half)
                              # t = raw + beta (bf16), roll via perm matmul
                              t = t_p.tile([128, 1024], BF16, tag="t")
                              nc.scalar.add(t[:], pq[:], bm(m))
                              pr = pr_ps.tile([128, 1024], F32, tag="pr")
                              for nn in range(2):
                                  nc.tensor.matmul(
                                      pr[:, nn * 512:(nn + 1) * 512], r2_sb[:],
                                      t[:, nn * 512:(nn + 1) * 512],
                                      start=True, stop=True)
                              # dst = t*cos + roll(t)*sin
                              t1 = t1_p.tile([128, 1024], BF16, tag="t1")
                              nc.gpsimd.tensor_tensor(
                                  t1[:], t[:], cos_sb[:, hs], op=OP.mult)
                              nc.vector.scalar_tensor_tensor(
                                  dst[:, hs], pr[:], 0.0, sin_sb[:, hs],
                                  op0=OP.add, op1=OP.mult)
                              nc.vector.tensor_tensor(
                                  dst[:, hs], dst[:, hs], t1[:], op=OP.add)
                          # v
                          m = 3 * p + 2
                          pv = pq_ps.tile([128, 1024], F32, tag="pq")
                          qkv_mm(pv, m, half)
                          nc.vector.tensor_scalar_add(vsb[:, hs], pv[:], bm(m))
                      # transpose v to ktok-major, 4 tiles per psum bank,
                      # one fused strided evac per group into the paired
                      # [v_h0|1|v_h1|1] layout.
                      for g in range(NT // 4):
                          ptV = ptV_ps.tile([128, 4, 128], BF16, tag="ptV")
                          for j in range(4):
                              mt = 4 * g + j
                              nc.tensor.transpose(
                                  ptV[:, j, :],
                                  vsb[:, mt * 128:(mt + 1) * 128], ident_bf[:])
                          dstv = vap[p][:, 4 * g:4 * g + 4, :].rearrange(
                              "p m (h d) -> p m h d", h=2, d=65)[:, :, :, 0:64]
                          nc.scalar.copy(
                              dstv, ptV.rearrange("p m (h d) -> p m h d",
                                                  h=2, d=64))

          # ---- Stage C: attention per pair (+ interleaved out-proj) ----
          with ExitStack() as s3:
              s_ps = s3.enter_context(tc.tile_pool(name="sps", bufs=2,
                                                   space="PSUM"))
              a_ps = s3.enter_context(tc.tile_pool(name="aps", bufs=1,
                                                   space="PSUM"))
              po_ps = s3.enter_context(tc.tile_pool(name="pops", bufs=2,
                                                    space="PSUM"))
              p_pool = s3.enter_context(tc.tile_pool(name="pp", bufs=2))
              nrm_p = s3.enter_context(tc.tile_pool(name="nrm", bufs=2))
              ye_p = s3.enter_context(tc.tile_pool(name="ye", bufs=3))

              scale = float(DH) ** -0.5

              def boundary_pieces(pp, pqc, pacc, pP, tail=False):
                  """Previous-qchunk epilogue as (target_mt, fn) pieces.
                  PE work is chopped into <=~0.45us pieces scheduled at
                  the mt where their inputs are ready, so the in-order
                  PE stream never blocks on the slow DVE reciprocal
                  (four [1,256] chunks, ~1.7us each). tail=True is the
                  final drain: latency-optimized (D row first on DVE,
                  acc evac + ye evacs on the now-idle ACT, fp32 bcast
                  quarters gated per recip chunk)."""
                  qsl_n = slice(pqc * W, (pqc + 1) * W)
                  ou16 = nrm_p.tile([64, 2, W], BF16, tag="ou16")
                  ds = nrm_p.tile([1, 2 * W], F32, tag="ds")
                  rds = nrm_p.tile([1, 2 * W], F32, tag="rds")
                  rds16 = nrm_p.tile([1, 2 * W], BF16, tag="rds16")
                  rbp = [None, None]

                  def pv_flush():
                      for hh in range(2):
                          nc.tensor.matmul(
                              pacc[0:65, hh, :],
                              vap[pp][:, NT - 1, hh * 65:(hh + 1) * 65],
                              pP[:, NT - 1, hh, :], start=False, stop=True)
                      if tail:
                          nc.scalar.copy(ou16[:], pacc[0:64, :, :])
                          nc.vector.tensor_copy(
                              ds[:], pacc[64:65, :, :].rearrange(
                                  "p a b -> p (a b)"))
                          for k in range(4):
                              ck = slice(k * 256, (k + 1) * 256)
                              nc.vector.reciprocal(rds[:, ck], ds[:, ck])
                          return
                      # evacuate numerators + D row (frees the acc psum
                      # banks), 1/D in 4 chunks, cast per half for the
                      # cheap bf16 broadcast matmuls.
                      nc.vector.tensor_copy(ou16[:], pacc[0:64, :, :])
                      nc.vector.tensor_copy(
                          ds[:], pacc[64:65, :, :].rearrange(
                              "p a b -> p (a b)"))
                      for hh in range(2):
                          for k in (0, 1):
                              ck = slice(hh * W + k * 256,
                                         hh * W + (k + 1) * 256)
                              nc.vector.reciprocal(rds[:, ck], ds[:, ck])
                          hs = slice(hh * W, (hh + 1) * W)
                          nc.vector.tensor_copy(rds16[:, hs], rds[:, hs])

                  def bcast16(hh):
                      def f():
                          rbp[hh] = po_ps.tile([128, W], F32, tag="po",
                                               name=f"rbp{hh}")
                          nc.tensor.matmul(
                              rbp[hh][0:64, :], ones_bf[:, :],
                              rds16[:, hh * W:(hh + 1) * W],
                              start=True, stop=True)
                      return f

                  def bcast32(k):
                      def f():
                          hh, half = k // 2, k % 2
                          if rbp[hh] is None:
                              rbp[hh] = po_ps.tile([128, W], F32, tag="po",
                                                   name=f"rbp{hh}")
                          cs = slice(half * 256, (half + 1) * 256)
                          nc.tensor.matmul(rbp[hh][0:64, cs], ones_f32[:, :],
                                           rds[:, k * 256:(k + 1) * 256],
                                           start=True, stop=True)
                      return f

                  def nmul():
                      for hh in range(2):
                          nc.vector.tensor_tensor(
                              outn[pp][hh * 64:hh * 64 + 64, qsl_n],
                              ou16[:, hh, :], rbp[hh][0:64, :], op=OP.mult)

                  def oproj(mi):
                      def f():
                          po = po_ps.tile([128, W], F32, tag="po",
                                          name=f"po{mi}")
                          for pr in range(2):
                              nc.tensor.matmul(
                                  po[:],
                                  wout_sb[:, pr, mi * 128:(mi + 1) * 128],
                                  outn[pr][:, qsl_n], start=(pr == 0),
                                  stop=(pr == 1))
                          ye = ye_p.tile([128, W], BF16, tag="ye")
                          if tail:
                              nc.scalar.copy(ye[:], po[:])
                          else:
                              nc.vector.tensor_copy(ye[:], po[:])
                          nc.sync.dma_start(
                              y[mi * 128:(mi + 1) * 128, qsl_n], ye[:])
                      return f

                  if tail:
                      pieces = [(1, pv_flush)]
                      pieces += [(2 + k, bcast32(k)) for k in range(4)]
                      pieces.append((6, nmul))
                      pieces += [(7 + mi, oproj(mi)) for mi in range(4)]
                      return pieces

                  pieces = [(1, pv_flush), (7, bcast16(0)), (11, bcast16(1)),
                            (12, nmul)]
                  if pp == 1:
                      pieces += [(13 + mi, oproj(mi)) for mi in range(4)]
                  return pieces

              pend = []
              for p in range(2):
                  for qc in range(N // W):
                      qsl = slice(qc * W, (qc + 1) * W)
                      P = p_pool.tile([128, NT, 2, W], BF16, name="P",
                                      tag="P")
                      acc = None
                      for mt in range(NT):
                          s3t = s_ps.tile([128, 2, W], F32, tag="s3")
                          for hh in range(2):
                              hsl = slice(hh * 64, hh * 64 + 64)
                              nc.tensor.matmul(
                                  s3t[:, hh, :],
                                  ks[p][hsl, mt * 128:(mt + 1) * 128],
                                  qs[p][hsl, qsl],
                                  start=True, stop=True)
                          nc.scalar.activation(
                              P[:, mt, :, :], s3t[:], ACT.Exp, scale=scale)
                          while pend and pend[0][0] <= mt:
                              pend.pop(0)[1]()
                          if mt == 1:
                              acc = a_ps.tile([128, 2, W], F32, name="acc",
                                              tag="acc")
                          if mt >= 1:
                              for hh in range(2):
                                  nc.tensor.matmul(
                                      acc[0:65, hh, :],
                                      vap[p][:, mt - 1, hh * 65:(hh + 1) * 65],
                                      P[:, mt - 1, hh, :],
                                      start=(mt - 1 == 0), stop=False)
                      while pend:
                          pend.pop(0)[1]()
                      pend = boundary_pieces(p, qc, acc, P,
                                             tail=(p == 1 and
                                                   qc == N // W - 1))
              while pend:
                  pend.pop(0)[1]()

    _split_multiwait(nc)
    return nc


def _host_prep(x, ln_gamma, ln_beta, w_qkv, w_out):
    """Build the 8 per-core input maps."""
    import ml_dtypes
    f32 = np.float32
    bf16 = ml_dtypes.bfloat16
    pos = np.arange(N, dtype=f32)[:, None]
    idx = np.arange(DH, dtype=f32)[None, :]
    angle = pos / (f32(10000.0) ** (idx / f32(DH)))       # [N, DH]
    cos2 = np.ascontiguousarray(np.tile(np.cos(angle).T, (2, 1))).astype(bf16)
    sin2 = np.ascontiguousarray(np.tile(np.sin(angle).T, (2, 1))).astype(bf16)
    ident = np.eye(128, dtype=f32)
    roll64 = np.zeros((64, 64), f32)
    for p in range(64):
        roll64[(p - 1) % 64, p] = 1.0     # lhsT[src, dst]: dst p <- src p-1
    r2 = np.zeros((128, 128), f32)
    r2[0:64, 0:64] = roll64
    r2[64:128, 64:128] = roll64
    r2 = r2.astype(bf16)

    wg = (w_qkv * ln_gamma[:, None]).astype(f32)          # [512, 1536]
    beta_row = (ln_beta @ w_qkv).astype(f32)              # [1536]

    def head_block(a, sec, h):    # sec 0=q 1=k 2=v, global head h
        return a[..., sec * 512 + h * DH: sec * 512 + (h + 1) * DH]

    in_maps = []
    for c in range(8):
        bi, hg = c // 2, c % 2
        hs = [4 * hg + i for i in range(HPC)]
        mts, bcols = [], []
        # M-tile order: k01 q01 v01 k23 q23 v23
        for pr in range(2):
            for sec in (1, 0, 2):
                mts.append(np.concatenate(
                    [head_block(wg, sec, hs[2 * pr]),
                     head_block(wg, sec, hs[2 * pr + 1])], axis=1))
                bcols.append(np.concatenate(
                    [head_block(beta_row, sec, hs[2 * pr]),
                     head_block(beta_row, sec, hs[2 * pr + 1])]))
        wqkv_c = np.ascontiguousarray(
            np.concatenate(mts, axis=1)).astype(bf16)     # [512, 768]
        beta_c = np.stack(bcols, axis=1).astype(f32)      # [128, 6]
        wout_c = np.ascontiguousarray(
            w_out[hg * 256:(hg + 1) * 256, :]).astype(bf16)
        in_maps.append({
            "x_nat": np.ascontiguousarray(x[bi]).astype(bf16),
            "wqkv": wqkv_c,
            "beta_mt": beta_c,
            "r2": r2,
            "wout": wout_c,
            "cos2": cos2,
            "sin2": sin2,
            "ident": ident,
        })
    return in_maps


_NC = None


def kernel(x, ln_gamma, ln_beta, w_qkv, w_out, b_out, **run_kwargs):
    global _NC
    x = np.asarray(x, dtype=np.float32)
    assert x.shape == (B, N, D), x.shape
    if _NC is None:
        _NC = build_nc()
    in_maps = _host_prep(np.asarray(x), np.asarray(ln_gamma),
                         np.asarray(ln_beta), np.asarray(w_qkv),
                         np.asarray(w_out))
    res = run_bass_kernel_spmd(_NC, in_maps, core_ids=list(range(8)), **run_kwargs)
    out = np.empty((B, N, D), dtype=np.float32)
    for bi in range(B):
        part = (res.results[2 * bi]["y"].astype(np.float32)
                + res.results[2 * bi + 1]["y"].astype(np.float32))
        out[bi] = part.T + np.asarray(b_out, dtype=np.float32)
    kernel.last_results = res
    return out


# revision 27
# speedup vs baseline: 1.0928x; 1.0928x over previous
"""Fused LN + QKV + RoPE + attention + out-proj Trainium2 kernel, v4.

Shapes (hardcoded from the problem spec):
  x [4, 2048, 512] fp32, w_qkv [512, 1536], w_out [512, 512],
  ln_gamma/ln_beta/b_out [512]. 8 heads of 64. Output [4, 2048, 512].

Sharding: 8 cores = 4 batches x 2 head-groups (4 heads each). Each core
computes a w_out row-split partial output for its batch; the host sums
the two partials per batch and adds b_out.

Design notes (ACT-exp is the roofline: 16.8M exp/core ~= 110us min):
 - LN is two-pass: per-tile sum (DVE reduce) and sum-of-squares, then
   ONE batched sqrt + reciprocal, then per-tile xn (bf16 so the PE
   transpose runs at 1 cycle/row).
 - QKV: 6 M-tiles (k/q/v per pair); RoPE's roll computed by a
   block-diagonal permutation matmul on (q + beta); combine split
   across GpSimd (t*cos) and DVE (pr*sin, final bf16 add in 2x mode).
 - Attention per head-pair, per-mt software pipeline: QK row-tiled
   2 heads concurrently in the 128x128 PE array into one [128,2,512]
   PSUM slab (2 rotating slabs), ONE exp per mt covering both heads so
   ACT runs back-to-back; PV (ones-augmented V, M=65, row 64 = softmax
   denominator) trails exp by one mt. PE order QK(mt+1) before PV(mt)
   so the in-order PE never blocks the exp chain.
 - Normalize: reciprocal_approx_fast (~5x faster than DVE RECIPROCAL)
   on both heads' D rows, fp32 ones-matmul broadcast into rows 64:128
   of the acc's own psum banks, one fused scalar_tensor_tensor
   (acc * 1/D) per head. Emitted at the START of the next qchunk.
 - Out-proj interleaved: emitted per-qchunk during pair 1's attention
   (needs both pairs' outn), evacuated on DVE (ACT is exp-saturated).
 - PSUM: s3 slabs 2x2 + acc 2 + out-proj 2 = 8 banks exactly.
Matmul operands bf16 (fp32 PSUM accumulation); LN/softmax math fp32.
"""

import numpy as np

import concourse.bass as bass
import concourse.tile as tile
from concourse import mybir
from concourse.bass_utils import run_bass_kernel_spmd

F32 = mybir.dt.float32
BF16 = mybir.dt.bfloat16
AX = mybir.AxisListType
OP = mybir.AluOpType
ACT = mybir.ActivationFunctionType

B, N, D = 4, 2048, 512
HEADS, DH = 8, 64
HPC = 4            # heads per core
EPS = 1e-5
NT = N // 128      # 16 token tiles
KT = D // 128      # 4 feature tiles
W = 512            # attention query-chunk width


def _split_multiwait(nc):
    """Insert NoOps so no instruction carries more than one sem wait.

    The pinned walrus rejects >1 sync wait per instruction
    (setupSyncWait "Too many sync wait commands"). Waits are a
    conjunction, so hoisting all but the last onto same-engine NoOps
    immediately before the instruction is equivalent.
    """
    ctr = 0
    for fn in nc.m.functions:
        for blk in fn.blocks:
            insts = blk.instructions
            idx = 0
            while idx < len(insts):
                inst = insts[idx]
                si = inst.sync_info
                if si is not None and len(si.on_wait) > 1:
                    waits = list(si.on_wait)
                    for w in waits[:-1]:
                        nop = mybir.InstNoOp(name=f"SWNOP-{ctr}", ins=[], outs=[])
                        ctr += 1
                        nop.engine = inst.engine
                        nop.sync_info = mybir.SyncInfo(on_wait=[w], on_update=[])
                        insts.insert(idx, nop)
                        idx += 1
                    inst.sync_info = mybir.SyncInfo(
                        on_wait=[waits[-1]], on_update=list(si.on_update)
                    )
                idx += 1


def build_nc(loops=1):
    from contextlib import ExitStack

    nc = bass.Bass("TRN2", target_bir_lowering=False, num_devices=8)

    x_nat = nc.dram_tensor("x_nat", [N, D], BF16, kind="ExternalInput")
    # gamma-folded QKV weights bf16, M-tile order k01 q01 v01 k23 q23 v23
    wqkv = nc.dram_tensor("wqkv", [D, 768], BF16, kind="ExternalInput")
    beta_mt = nc.dram_tensor("beta_mt", [128, 6], F32, kind="ExternalInput")
    r2 = nc.dram_tensor("r2", [128, 128], BF16, kind="ExternalInput")
    wout = nc.dram_tensor("wout", [HPC * DH, D], BF16, kind="ExternalInput")
    cos2 = nc.dram_tensor("cos2", [128, N], BF16, kind="ExternalInput")
    sin2 = nc.dram_tensor("sin2", [128, N], BF16, kind="ExternalInput")
    ident = nc.dram_tensor("ident", [128, 128], F32, kind="ExternalInput")
    y = nc.dram_tensor("y", [D, N], BF16, kind="ExternalOutput")

    with tile.TileContext(nc) as tc:
      for _loop in range(loops):
        with ExitStack() as ctx:
          const = ctx.enter_context(tc.tile_pool(name="const", bufs=1))
          qk_pool = ctx.enter_context(tc.tile_pool(name="qk", bufs=1))
          va_pool = ctx.enter_context(tc.tile_pool(name="va", bufs=1))
          outn_pool = ctx.enter_context(tc.tile_pool(name="outn", bufs=1))

          ident_sb = const.tile([128, 128], F32)
          nc.gpsimd.dma_start(ident_sb[:], ident[:, :])
          ident_bf = const.tile([128, 128], BF16)
          nc.vector.tensor_copy(ident_bf[:], ident_sb[:])
          r2_sb = const.tile([128, 128], BF16)
          nc.gpsimd.dma_start(r2_sb[:], r2[:, :])
          eps_sb = const.tile([128, 1], F32)
          nc.vector.memset(eps_sb[:], EPS)
          ones_f32 = const.tile([1, 64], F32)
          nc.vector.memset(ones_f32[:], 1.0)
          ones_bf = const.tile([1, 64], BF16)
          nc.vector.memset(ones_bf[:], 1.0)
          beta_sb = const.tile([128, 6], F32)
          nc.gpsimd.dma_start(beta_sb[:], beta_mt[:, :])
          wq_sb = const.tile([128, KT, 768], BF16, name="wq")
          for kt in range(KT):
              nc.gpsimd.dma_start(wq_sb[:, kt, :],
                                  wqkv[kt * 128:(kt + 1) * 128, :])
          wout_sb = const.tile([128, 2, D], BF16, name="wout")
          for p in range(2):
              nc.scalar.dma_start(wout_sb[:, p, :],
                                  wout[p * 128:(p + 1) * 128, :])
          cos_sb = const.tile([128, N], BF16, name="cos")
          nc.scalar.dma_start(cos_sb[:], cos2[:, :])
          sin_sb = const.tile([128, N], BF16, name="sin")
          nc.scalar.dma_start(sin_sb[:], sin2[:, :])

          # q/k rope'd feature-major per pair [128, N]; vap per pair holds
          # both heads' V ktok-major with ones columns at 64 and 129.
          qs = [qk_pool.tile([128, N], BF16, name=f"qs{p}", tag=f"qs{p}")
                for p in range(2)]
          ks = [qk_pool.tile([128, N], BF16, name=f"ks{p}", tag=f"ks{p}")
                for p in range(2)]
          vap = [va_pool.tile([128, NT, 130], BF16, name=f"vap{p}",
                              tag=f"vap{p}") for p in range(2)]
          for p in range(2):
              nc.vector.memset(vap[p][:], 1.0)
          outn = [outn_pool.tile([128, N], BF16, name=f"on{p}", tag=f"on{p}")
                  for p in range(2)]

          # ---- Stage A: LayerNorm (two-pass) + PE transpose ----
          with ExitStack() as s1:
              x_p = s1.enter_context(tc.tile_pool(name="x", bufs=1))
              st_p = s1.enter_context(tc.tile_pool(name="st", bufs=1))
              xn_p = s1.enter_context(tc.tile_pool(name="xn", bufs=3))
              scr_p = s1.enter_context(tc.tile_pool(name="scr", bufs=2))
              xnT_p = s1.enter_context(tc.tile_pool(name="xnT", bufs=1))
              ptA_ps = s1.enter_context(tc.tile_pool(name="ptA", bufs=1,
                                                     space="PSUM"))

              xts = x_p.tile([128, NT, D], BF16, name="xts")
              muvar = st_p.tile([128, NT, 2], F32, name="muvar")
              for tt in range(NT):
                  nc.sync.dma_start(xts[:, tt, :],
                                    x_nat[tt * 128:(tt + 1) * 128, :])
                  bn6 = scr_p.tile([128, 6], F32, tag="bn6")
                  nc.vector.bn_stats(bn6[:], xts[:, tt, :])
                  nc.vector.bn_aggr(muvar[:, tt, :], bn6[:])
              mu_all = muvar[:, :, 0:1].rearrange("p a b -> p (a b)")
              sd_all = st_p.tile([128, NT], F32, name="sd_all")
              nc.scalar.activation(sd_all[:],
                                   muvar[:, :, 1:2].rearrange(
                                       "p a b -> p (a b)"),
                                   ACT.Sqrt, bias=eps_sb[:])
              rs_all = st_p.tile([128, NT], F32, name="rs_all")
              nc.vector.reciprocal(rs_all[:], sd_all[:])
              bias2 = st_p.tile([128, NT], F32, name="bias2")
              nc.vector.scalar_tensor_tensor(
                  bias2[:], mu_all[:], -1.0, rs_all[:], op0=OP.mult,
                  op1=OP.mult)

              xnT = xnT_p.tile([128, KT, N], BF16, name="xnT")
              for tt in range(NT):
                  xn = xn_p.tile([128, D], BF16, tag="xn")
                  if tt % 2 == 0:
                      nc.scalar.activation(xn[:], xts[:, tt, :], ACT.Identity,
                                           bias=bias2[:, tt:tt + 1],
                                           scale=rs_all[:, tt:tt + 1])
                  else:
                      nc.vector.tensor_scalar(
                          xn[:], xts[:, tt, :], muvar[:, tt, 0:1],
                          rs_all[:, tt:tt + 1], op0=OP.subtract, op1=OP.mult)
                  pt = ptA_ps.tile([128, KT, 128], BF16, tag="pt")
                  for ft in range(KT):
                      nc.tensor.transpose(
                          pt[:, ft, :], xn[:, ft * 128:(ft + 1) * 128],
                          ident_bf[:])
                  nc.scalar.copy(
                      xnT[:, :, tt * 128:(tt + 1) * 128], pt[:])

              # ---- Stage B: QKV + RoPE per pair ----
              with ExitStack() as s2:
                  pq_ps = s2.enter_context(tc.tile_pool(name="pq", bufs=2,
                                                        space="PSUM"))
                  pr_ps = s2.enter_context(tc.tile_pool(name="pr", bufs=1,
                                                        space="PSUM"))
                  ptV_ps = s2.enter_context(tc.tile_pool(name="ptV", bufs=1,
                                                         space="PSUM"))
                  t_p = s2.enter_context(tc.tile_pool(name="t", bufs=3))
                  t1_p = s2.enter_context(tc.tile_pool(name="t1", bufs=2))
                  vsb_p = s2.enter_context(tc.tile_pool(name="vsb", bufs=2))

                  def bm(m):
                      return beta_sb[:, m:m + 1]

                  def qkv_mm(psum_ap, m, half):
                      ms = slice(m * 128, (m + 1) * 128)
                      for nn in range(2):
                          cs = slice(half * 1024 + nn * 512,
                                     half * 1024 + (nn + 1) * 512)
                          for kt in range(KT):
                              nc.tensor.matmul(
                                  psum_ap[:, nn * 512:(nn + 1) * 512],
                                  wq_sb[:, kt, ms], xnT[:, kt, cs],
                                  start=(kt == 0), stop=(kt == KT - 1))

                  for p in range(2):
                      vsb = vsb_p.tile([128, N], BF16, tag=f"vsb{p}")
                      for half in range(2):
                          hs = slice(half * 1024, (half + 1) * 1024)
                          for sec, dst in ((0, ks[p]), (1, qs[p])):
                              m = 3 * p + sec
                              pq = pq_ps.tile([128, 1024], F32, tag="pq")
                              qkv_mm(pq, m, half)
                              # t = raw + beta (bf16), roll via perm matmul
                              t = t_p.tile([128, 1024], BF16, tag="t")
                              nc.scalar.add(t[:], pq[:], bm(m))
                              pr = pr_ps.tile([128, 1024], F32, tag="pr")
                              for nn in range(2):
                                  nc.tensor.matmul(
                                      pr[:, nn * 512:(nn + 1) * 512], r2_sb[:],
                                      t[:, nn * 512:(nn + 1) * 512],
                                      start=True, stop=True)
                              # dst = t*cos + roll(t)*sin
                              t1 = t1_p.tile([128, 1024], BF16, tag="t1")
                              nc.gpsimd.tensor_tensor(
                                  t1[:], t[:], cos_sb[:, hs], op=OP.mult)
                              nc.vector.scalar_tensor_tensor(
                                  dst[:, hs], pr[:], 0.0, sin_sb[:, hs],
                                  op0=OP.add, op1=OP.mult)
                              nc.vector.tensor_tensor(
                                  dst[:, hs], dst[:, hs], t1[:], op=OP.add)
                          # v
                          m = 3 * p + 2
                          pv = pq_ps.tile([128, 1024], F32, tag="pq")
                          qkv_mm(pv, m, half)
                          nc.vector.tensor_scalar_add(vsb[:, hs], pv[:], bm(m))
                      # transpose v to ktok-major, 4 tiles per psum bank,
                      # one fused strided evac per group into the paired
                      # [v_h0|1|v_h1|1] layout.
                      for g in range(NT // 4):
                          ptV = ptV_ps.tile([128, 4, 128], BF16, tag="ptV")
                          for j in range(4):
                              mt = 4 * g + j
                              nc.tensor.transpose(
                                  ptV[:, j, :],
                                  vsb[:, mt * 128:(mt + 1) * 128], ident_bf[:])
                          dstv = vap[p][:, 4 * g:4 * g + 4, :].rearrange(
                              "p m (h d) -> p m h d", h=2, d=65)[:, :, :, 0:64]
                          nc.scalar.copy(
                              dstv, ptV.rearrange("p m (h d) -> p m h d",
                                                  h=2, d=64))

          # ---- Stage C: attention per pair (+ interleaved out-proj) ----
          with ExitStack() as s3:
              s_ps = s3.enter_context(tc.tile_pool(name="sps", bufs=2,
                                                   space="PSUM"))
              a_ps = s3.enter_context(tc.tile_pool(name="aps", bufs=1,
                                                   space="PSUM"))
              po_ps = s3.enter_context(tc.tile_pool(name="pops", bufs=2,
                                                    space="PSUM"))
              p_pool = s3.enter_context(tc.tile_pool(name="pp", bufs=2))
              nrm_p = s3.enter_context(tc.tile_pool(name="nrm", bufs=2))
              ye_p = s3.enter_context(tc.tile_pool(name="ye", bufs=3))

              scale = float(DH) ** -0.5

              def boundary_pieces(pp, pqc, pacc, pP, tail=False):
                  """Previous-qchunk epilogue as (target_mt, fn) pieces.
                  PE work is chopped into <=~0.45us pieces scheduled at
                  the mt where their inputs are ready, so the in-order
                  PE stream never blocks on the slow DVE reciprocal
                  (four [1,256] chunks, ~1.7us each). tail=True is the
                  final drain: latency-optimized (D row first on DVE,
                  acc evac + ye evacs on the now-idle ACT, fp32 bcast
                  quarters gated per recip chunk)."""
                  qsl_n = slice(pqc * W, (pqc + 1) * W)
                  ou16 = nrm_p.tile([64, 2, W], BF16, tag="ou16")
                  ds = nrm_p.tile([1, 2 * W], F32, tag="ds")
                  rds = nrm_p.tile([1, 2 * W], F32, tag="rds")
                  rds16 = nrm_p.tile([1, 2 * W], BF16, tag="rds16")
                  rbp = [None, None]

                  def pv_flush():
                      for hh in range(2):
                          nc.tensor.matmul(
                              pacc[0:65, hh, :],
                              vap[pp][:, NT - 1, hh * 65:(hh + 1) * 65],
                              pP[:, NT - 1, hh, :], start=False, stop=True)
                      if tail:
                          nc.scalar.copy(ou16[:], pacc[0:64, :, :])
                          nc.vector.tensor_copy(
                              ds[:], pacc[64:65, :, :].rearrange(
                                  "p a b -> p (a b)"))
                          for k in range(4):
                              ck = slice(k * 256, (k + 1) * 256)
                              nc.vector.reciprocal(rds[:, ck], ds[:, ck])
                          return
                      # evacuate numerators + D row (frees the acc psum
                      # banks), then 1/D in 4 chunks.
                      nc.vector.tensor_copy(ou16[:], pacc[0:64, :, :])
                      nc.vector.tensor_copy(
                          ds[:], pacc[64:65, :, :].rearrange(
                              "p a b -> p (a b)"))
                      for k in range(4):
                          ck = slice(k * 256, (k + 1) * 256)
                          nc.vector.reciprocal(rds[:, ck], ds[:, ck])

                  def bcast32(k):
                      def f():
                          hh, half = k // 2, k % 2
                          if rbp[hh] is None:
                              rbp[hh] = po_ps.tile([128, W], F32, tag="po",
                                                   name=f"rbp{hh}")
                          cs = slice(half * 256, (half + 1) * 256)
                          nc.tensor.matmul(rbp[hh][0:64, cs], ones_f32[:, :],
                                           rds[:, k * 256:(k + 1) * 256],
                                           start=True, stop=True)
                      return f

                  def nmul():
                      for hh in range(2):
                          nc.vector.tensor_tensor(
                              outn[pp][hh * 64:hh * 64 + 64, qsl_n],
                              ou16[:, hh, :], rbp[hh][0:64, :], op=OP.mult)

                  def oproj(mi):
                      def f():
                          po = po_ps.tile([128, W], F32, tag="po",
                                          name=f"po{mi}")
                          for pr in range(2):
                              nc.tensor.matmul(
                                  po[:],
                                  wout_sb[:, pr, mi * 128:(mi + 1) * 128],
                                  outn[pr][:, qsl_n], start=(pr == 0),
                                  stop=(pr == 1))
                          ye = ye_p.tile([128, W], BF16, tag="ye")
                          if tail:
                              nc.scalar.copy(ye[:], po[:])
                          else:
                              nc.vector.tensor_copy(ye[:], po[:])
                          nc.sync.dma_start(
                              y[mi * 128:(mi + 1) * 128, qsl_n], ye[:])
                      return f

                  if tail:
                      pieces = [(1, pv_flush)]
                      pieces += [(2 + k, bcast32(k)) for k in range(4)]
                      pieces.append((6, nmul))
                      pieces += [(7 + mi, oproj(mi)) for mi in range(4)]
                      return pieces

                  pieces = [(1, pv_flush), (5, bcast32(0)), (6, bcast32(1)),
                            (8, bcast32(2)), (10, bcast32(3)), (11, nmul)]
                  if pp == 1:
                      pieces += [(12 + mi, oproj(mi)) for mi in range(4)]
                  return pieces

              pend = []
              for p in range(2):
                  for qc in range(N // W):
                      qsl = slice(qc * W, (qc + 1) * W)
                      P = p_pool.tile([128, NT, 2, W], BF16, name="P",
                                      tag="P")
                      acc = None
                      pv_done = 0    # PVs deferred past the acc-release
                      for mt in range(NT):
                          s3t = s_ps.tile([128, 2, W], F32, tag="s3")
                          for hh in range(2):
                              hsl = slice(hh * 64, hh * 64 + 64)
                              nc.tensor.matmul(
                                  s3t[:, hh, :],
                                  ks[p][hsl, mt * 128:(mt + 1) * 128],
                                  qs[p][hsl, qsl],
                                  start=True, stop=True)
                          nc.scalar.activation(
                              P[:, mt, :, :], s3t[:], ACT.Exp, scale=scale)
                          while pend and pend[0][0] <= mt:
                              pend.pop(0)[1]()
                          if mt == 1:
                              acc = a_ps.tile([128, 2, W], F32, name="acc",
                                              tag="acc")
                          # PV stream: none at mts 1-3 (the prev qchunk's
                          # acc evac hasn't freed the banks yet; a parked
                          # PV would block the in-order PE), then catch
                          # up two per mt.
                          if mt >= 4:
                              tgt = min(mt, NT - 1)
                              while pv_done < tgt:
                                  j = pv_done
                                  for hh in range(2):
                                      nc.tensor.matmul(
                                          acc[0:65, hh, :],
                                          vap[p][:, j,
                                                 hh * 65:(hh + 1) * 65],
                                          P[:, j, hh, :],
                                          start=(j == 0), stop=False)
                                  pv_done += 1
                                  if pv_done >= min(2 * (mt - 3), tgt):
                                      break
                      while pend:
                          pend.pop(0)[1]()
                      pend = boundary_pieces(p, qc, acc, P,
                                             tail=(p == 1 and
                                                   qc == N // W - 1))
              while pend:
                  pend.pop(0)[1]()

    _split_multiwait(nc)
    return nc


def _host_prep(x, ln_gamma, ln_beta, w_qkv, w_out):
    """Build the 8 per-core input maps."""
    import ml_dtypes
    f32 = np.float32
    bf16 = ml_dtypes.bfloat16
    pos = np.arange(N, dtype=f32)[:, None]
    idx = np.arange(DH, dtype=f32)[None, :]
    angle = pos / (f32(10000.0) ** (idx / f32(DH)))       # [N, DH]
    cos2 = np.ascontiguousarray(np.tile(np.cos(angle).T, (2, 1))).astype(bf16)
    sin2 = np.ascontiguousarray(np.tile(np.sin(angle).T, (2, 1))).astype(bf16)
    ident = np.eye(128, dtype=f32)
    roll64 = np.zeros((64, 64), f32)
    for p in range(64):
        roll64[(p - 1) % 64, p] = 1.0     # lhsT[src, dst]: dst p <- src p-1
    r2 = np.zeros((128, 128), f32)
    r2[0:64, 0:64] = roll64
    r2[64:128, 64:128] = roll64
    r2 = r2.astype(bf16)

    wg = (w_qkv * ln_gamma[:, None]).astype(f32)          # [512, 1536]
    beta_row = (ln_beta @ w_qkv).astype(f32)              # [1536]

    def head_block(a, sec, h):    # sec 0=q 1=k 2=v, global head h
        return a[..., sec * 512 + h * DH: sec * 512 + (h + 1) * DH]

    in_maps = []
    for c in range(8):
        bi, hg = c // 2, c % 2
        hs = [4 * hg + i for i in range(HPC)]
        mts, bcols = [], []
        # M-tile order: k01 q01 v01 k23 q23 v23
        for pr in range(2):
            for sec in (1, 0, 2):
                mts.append(np.concatenate(
                    [head_block(wg, sec, hs[2 * pr]),
                     head_block(wg, sec, hs[2 * pr + 1])], axis=1))
                bcols.append(np.concatenate(
                    [head_block(beta_row, sec, hs[2 * pr]),
                     head_block(beta_row, sec, hs[2 * pr + 1])]))
        wqkv_c = np.ascontiguousarray(
            np.concatenate(mts, axis=1)).astype(bf16)     # [512, 768]
        beta_c = np.stack(bcols, axis=1).astype(f32)      # [128, 6]
        wout_c = np.ascontiguousarray(
            w_out[hg * 256:(hg + 1) * 256, :]).astype(bf16)
        in_maps.append({
            "x_nat": np.ascontiguousarray(x[bi]).astype(bf16),
            "wqkv": wqkv_c,
            "beta_mt": beta_c,
            "r2": r2,
            "wout": wout_c,
            "cos2": cos2,
            "sin2": sin2,
            "ident": ident,
        })
    return in_maps


_NC = None


def kernel(x, ln_gamma, ln_beta, w_qkv, w_out, b_out, **run_kwargs):
    global _NC
    x = np.asarray(x, dtype=np.float32)
    assert x.shape == (B, N, D), x.shape
    if _NC is None:
        _NC = build_nc()
    in_maps = _host_prep(np.asarray(x), np.asarray(ln_gamma),
                         np.asarray(ln_beta), np.asarray(w_qkv),
                         np.asarray(w_out))
    res = run_bass_kernel_spmd(_NC, in_maps, core_ids=list(range(8)), **run_kwargs)
    out = np.empty((B, N, D), dtype=np.float32)
    for bi in range(B):
        part = (res.results[2 * bi]["y"].astype(np.float32)
                + res.results[2 * bi + 1]["y"].astype(np.float32))
        out[bi] = part.T + np.asarray(b_out, dtype=np.float32)
    kernel.last_results = res
    return out


# revision 29
# speedup vs baseline: 1.1099x; 1.0157x over previous
"""Fused LN + QKV + RoPE + attention + out-proj Trainium2 kernel, v4.

Shapes (hardcoded from the problem spec):
  x [4, 2048, 512] fp32, w_qkv [512, 1536], w_out [512, 512],
  ln_gamma/ln_beta/b_out [512]. 8 heads of 64. Output [4, 2048, 512].

Sharding: 8 cores = 4 batches x 2 head-groups (4 heads each). Each core
computes a w_out row-split partial output for its batch; the host sums
the two partials per batch and adds b_out.

Design notes (ACT-exp is the roofline: 16.8M exp/core ~= 110us min):
 - LN is two-pass: per-tile sum (DVE reduce) and sum-of-squares, then
   ONE batched sqrt + reciprocal, then per-tile xn (bf16 so the PE
   transpose runs at 1 cycle/row).
 - QKV: 6 M-tiles (k/q/v per pair); RoPE's roll computed by a
   block-diagonal permutation matmul on (q + beta); combine split
   across GpSimd (t*cos) and DVE (pr*sin, final bf16 add in 2x mode).
 - Attention per head-pair, per-mt software pipeline: QK row-tiled
   2 heads concurrently in the 128x128 PE array into one [128,2,512]
   PSUM slab (2 rotating slabs), ONE exp per mt covering both heads so
   ACT runs back-to-back; PV (ones-augmented V, M=65, row 64 = softmax
   denominator) trails exp by one mt. PE order QK(mt+1) before PV(mt)
   so the in-order PE never blocks the exp chain.
 - Normalize: reciprocal_approx_fast (~5x faster than DVE RECIPROCAL)
   on both heads' D rows, fp32 ones-matmul broadcast into rows 64:128
   of the acc's own psum banks, one fused scalar_tensor_tensor
   (acc * 1/D) per head. Emitted at the START of the next qchunk.
 - Out-proj interleaved: emitted per-qchunk during pair 1's attention
   (needs both pairs' outn), evacuated on DVE (ACT is exp-saturated).
 - PSUM: s3 slabs 2x2 + acc 2 + out-proj 2 = 8 banks exactly.
Matmul operands bf16 (fp32 PSUM accumulation); LN/softmax math fp32.
"""

import numpy as np

import concourse.bass as bass
import concourse.tile as tile
from concourse import mybir
from concourse.bass_utils import run_bass_kernel_spmd

F32 = mybir.dt.float32
BF16 = mybir.dt.bfloat16
AX = mybir.AxisListType
OP = mybir.AluOpType
ACT = mybir.ActivationFunctionType

B, N, D = 4, 2048, 512
HEADS, DH = 8, 64
HPC = 4            # heads per core
EPS = 1e-5
NT = N // 128      # 16 token tiles
KT = D // 128      # 4 feature tiles
W = 512            # attention query-chunk width


def _split_multiwait(nc):
    """Insert NoOps so no instruction carries more than one sem wait.

    The pinned walrus rejects >1 sync wait per instruction
    (setupSyncWait "Too many sync wait commands"). Waits are a
    conjunction, so hoisting all but the last onto same-engine NoOps
    immediately before the instruction is equivalent.
    """
    ctr = 0
    for fn in nc.m.functions:
        for blk in fn.blocks:
            insts = blk.instructions
            idx = 0
            while idx < len(insts):
                inst = insts[idx]
                si = inst.sync_info
                if si is not None and len(si.on_wait) > 1:
                    waits = list(si.on_wait)
                    for w in waits[:-1]:
                        nop = mybir.InstNoOp(name=f"SWNOP-{ctr}", ins=[], outs=[])
                        ctr += 1
                        nop.engine = inst.engine
                        nop.sync_info = mybir.SyncInfo(on_wait=[w], on_update=[])
                        insts.insert(idx, nop)
                        idx += 1
                    inst.sync_info = mybir.SyncInfo(
                        on_wait=[waits[-1]], on_update=list(si.on_update)
                    )
                idx += 1


def build_nc(loops=1):
    from contextlib import ExitStack

    nc = bass.Bass("TRN2", target_bir_lowering=False, num_devices=8)

    x_nat = nc.dram_tensor("x_nat", [N, D], BF16, kind="ExternalInput")
    # gamma-folded QKV weights bf16, M-tile order k01 q01 v01 k23 q23 v23
    wqkv = nc.dram_tensor("wqkv", [D, 768], BF16, kind="ExternalInput")
    beta_mt = nc.dram_tensor("beta_mt", [128, 6], F32, kind="ExternalInput")
    r2 = nc.dram_tensor("r2", [128, 128], BF16, kind="ExternalInput")
    wout = nc.dram_tensor("wout", [HPC * DH, D], BF16, kind="ExternalInput")
    cos2 = nc.dram_tensor("cos2", [128, N], BF16, kind="ExternalInput")
    sin2 = nc.dram_tensor("sin2", [128, N], BF16, kind="ExternalInput")
    ident = nc.dram_tensor("ident", [128, 128], F32, kind="ExternalInput")
    y = nc.dram_tensor("y", [D, N], BF16, kind="ExternalOutput")

    with tile.TileContext(nc) as tc:
      for _loop in range(loops):
        with ExitStack() as ctx:
          const = ctx.enter_context(tc.tile_pool(name="const", bufs=1))
          qk_pool = ctx.enter_context(tc.tile_pool(name="qk", bufs=1))
          va_pool = ctx.enter_context(tc.tile_pool(name="va", bufs=1))
          outn_pool = ctx.enter_context(tc.tile_pool(name="outn", bufs=1))

          ident_sb = const.tile([128, 128], F32)
          nc.gpsimd.dma_start(ident_sb[:], ident[:, :])
          ident_bf = const.tile([128, 128], BF16)
          nc.vector.tensor_copy(ident_bf[:], ident_sb[:])
          r2_sb = const.tile([128, 128], BF16)
          nc.gpsimd.dma_start(r2_sb[:], r2[:, :])
          eps_sb = const.tile([128, 1], F32)
          nc.vector.memset(eps_sb[:], EPS)
          ones_f32 = const.tile([1, 64], F32)
          nc.vector.memset(ones_f32[:], 1.0)
          ones_bf = const.tile([1, 64], BF16)
          nc.vector.memset(ones_bf[:], 1.0)
          beta_sb = const.tile([128, 6], F32)
          nc.gpsimd.dma_start(beta_sb[:], beta_mt[:, :])
          wq_sb = const.tile([128, KT, 768], BF16, name="wq")
          for kt in range(KT):
              nc.gpsimd.dma_start(wq_sb[:, kt, :],
                                  wqkv[kt * 128:(kt + 1) * 128, :])
          wout_sb = const.tile([128, 2, D], BF16, name="wout")
          for p in range(2):
              nc.scalar.dma_start(wout_sb[:, p, :],
                                  wout[p * 128:(p + 1) * 128, :])
          cos_sb = const.tile([128, N], BF16, name="cos")
          nc.scalar.dma_start(cos_sb[:], cos2[:, :])
          sin_sb = const.tile([128, N], BF16, name="sin")
          nc.scalar.dma_start(sin_sb[:], sin2[:, :])

          # q/k rope'd feature-major per pair [128, N]; vap per pair holds
          # both heads' V ktok-major with ones columns at 64 and 129.
          qs = [qk_pool.tile([128, N], BF16, name=f"qs{p}", tag=f"qs{p}")
                for p in range(2)]
          ks = [qk_pool.tile([128, N], BF16, name=f"ks{p}", tag=f"ks{p}")
                for p in range(2)]
          vap = [va_pool.tile([128, NT, 130], BF16, name=f"vap{p}",
                              tag=f"vap{p}") for p in range(2)]
          for p in range(2):
              nc.vector.memset(vap[p][:], 1.0)
          outn = [outn_pool.tile([128, N], BF16, name=f"on{p}", tag=f"on{p}")
                  for p in range(2)]

          # ---- Stage A: LayerNorm (two-pass) + PE transpose ----
          with ExitStack() as s1:
              x_p = s1.enter_context(tc.tile_pool(name="x", bufs=1))
              st_p = s1.enter_context(tc.tile_pool(name="st", bufs=1))
              xn_p = s1.enter_context(tc.tile_pool(name="xn", bufs=3))
              scr_p = s1.enter_context(tc.tile_pool(name="scr", bufs=2))
              xnT_p = s1.enter_context(tc.tile_pool(name="xnT", bufs=1))
              ptA_ps = s1.enter_context(tc.tile_pool(name="ptA", bufs=1,
                                                     space="PSUM"))

              xts = x_p.tile([128, NT, D], BF16, name="xts")
              muvar = st_p.tile([128, NT, 2], F32, name="muvar")
              # x loads split across the sync and gpsimd DMA queues so
              # the tiles land twice as fast.
              for tt in range(NT):
                  eng = nc.sync if tt % 2 == 0 else nc.gpsimd
                  eng.dma_start(xts[:, tt, :],
                                x_nat[tt * 128:(tt + 1) * 128, :])
              mu_all = muvar[:, :, 0:1].rearrange("p a b -> p (a b)")
              var_all = muvar[:, :, 1:2].rearrange("p a b -> p (a b)")
              sd_all = st_p.tile([128, NT], F32, name="sd_all")
              rs_all = st_p.tile([128, NT], F32, name="rs_all")
              bias2 = st_p.tile([128, NT], F32, name="bias2")
              xnT = xnT_p.tile([128, KT, N], BF16, name="xnT")

              # batches of 4 tiles: stats -> batch sqrt/recip -> xn +
              # transpose + evac, so pass 2 of batch b overlaps pass 1
              # of batch b+1 instead of waiting for all 16 tiles.
              for b in range(NT // 4):
                  bs = slice(4 * b, 4 * b + 4)
                  for j in range(4):
                      tt = 4 * b + j
                      bn6 = scr_p.tile([128, 6], F32, tag="bn6")
                      nc.vector.bn_stats(bn6[:], xts[:, tt, :])
                      nc.vector.bn_aggr(muvar[:, tt, :], bn6[:])
                  nc.scalar.activation(sd_all[:, bs], var_all[:, bs],
                                       ACT.Sqrt, bias=eps_sb[:])
                  nc.vector.reciprocal(rs_all[:, bs], sd_all[:, bs])
                  nc.vector.scalar_tensor_tensor(
                      bias2[:, bs], mu_all[:, bs], -1.0, rs_all[:, bs],
                      op0=OP.mult, op1=OP.mult)
                  for j in range(4):
                      tt = 4 * b + j
                      xn = xn_p.tile([128, D], BF16, tag="xn")
                      if tt % 2 == 0:
                          nc.scalar.activation(
                              xn[:], xts[:, tt, :], ACT.Identity,
                              bias=bias2[:, tt:tt + 1],
                              scale=rs_all[:, tt:tt + 1])
                      else:
                          nc.vector.tensor_scalar(
                              xn[:], xts[:, tt, :], muvar[:, tt, 0:1],
                              rs_all[:, tt:tt + 1], op0=OP.subtract,
                              op1=OP.mult)
                      pt = ptA_ps.tile([128, KT, 128], BF16, tag="pt")
                      for ft in range(KT):
                          nc.tensor.transpose(
                              pt[:, ft, :], xn[:, ft * 128:(ft + 1) * 128],
                              ident_bf[:])
                      if tt % 2 == 0:
                          nc.vector.tensor_copy(
                              xnT[:, :, tt * 128:(tt + 1) * 128], pt[:])
                      else:
                          nc.scalar.copy(
                              xnT[:, :, tt * 128:(tt + 1) * 128], pt[:])

              # ---- Stage B: QKV + RoPE per pair ----
              with ExitStack() as s2:
                  pq_ps = s2.enter_context(tc.tile_pool(name="pq", bufs=2,
                                                        space="PSUM"))
                  pr_ps = s2.enter_context(tc.tile_pool(name="pr", bufs=1,
                                                        space="PSUM"))
                  ptV_ps = s2.enter_context(tc.tile_pool(name="ptV", bufs=1,
                                                         space="PSUM"))
                  t_p = s2.enter_context(tc.tile_pool(name="t", bufs=3))
                  t1_p = s2.enter_context(tc.tile_pool(name="t1", bufs=2))
                  vsb_p = s2.enter_context(tc.tile_pool(name="vsb", bufs=2))

                  def bm(m):
                      return beta_sb[:, m:m + 1]

                  def qkv_mm(psum_ap, m, half):
                      ms = slice(m * 128, (m + 1) * 128)
                      for nn in range(2):
                          cs = slice(half * 1024 + nn * 512,
                                     half * 1024 + (nn + 1) * 512)
                          for kt in range(KT):
                              nc.tensor.matmul(
                                  psum_ap[:, nn * 512:(nn + 1) * 512],
                                  wq_sb[:, kt, ms], xnT[:, kt, cs],
                                  start=(kt == 0), stop=(kt == KT - 1))

                  for p in range(2):
                      vsb = vsb_p.tile([128, N], BF16, tag=f"vsb{p}")
                      for half in range(2):
                          hs = slice(half * 1024, (half + 1) * 1024)
                          for sec, dst in ((0, ks[p]), (1, qs[p])):
                              m = 3 * p + sec
                              pq = pq_ps.tile([128, 1024], F32, tag="pq")
                              qkv_mm(pq, m, half)
                              # t = raw + beta (bf16), roll via perm matmul
                              t = t_p.tile([128, 1024], BF16, tag="t")
                              nc.scalar.add(t[:], pq[:], bm(m))
                              pr = pr_ps.tile([128, 1024], F32, tag="pr")
                              for nn in range(2):
                                  nc.tensor.matmul(
                                      pr[:, nn * 512:(nn + 1) * 512], r2_sb[:],
                                      t[:, nn * 512:(nn + 1) * 512],
                                      start=True, stop=True)
                              # dst = t*cos + roll(t)*sin; t*cos
                              # alternates GpSimd/DVE to balance load
                              t1 = t1_p.tile([128, 1024], BF16, tag="t1")
                              t1e = nc.gpsimd if (half + sec) % 2 else \
                                  nc.vector
                              t1e.tensor_tensor(
                                  t1[:], t[:], cos_sb[:, hs], op=OP.mult)
                              nc.vector.scalar_tensor_tensor(
                                  dst[:, hs], pr[:], 0.0, sin_sb[:, hs],
                                  op0=OP.add, op1=OP.mult)
                              nc.vector.tensor_tensor(
                                  dst[:, hs], dst[:, hs], t1[:], op=OP.add)
                          # v
                          m = 3 * p + 2
                          pv = pq_ps.tile([128, 1024], F32, tag="pq")
                          qkv_mm(pv, m, half)
                          nc.vector.tensor_scalar_add(vsb[:, hs], pv[:], bm(m))
                      # transpose v to ktok-major, 4 tiles per psum bank,
                      # one fused strided evac per group into the paired
                      # [v_h0|1|v_h1|1] layout.
                      for g in range(NT // 4):
                          ptV = ptV_ps.tile([128, 4, 128], BF16, tag="ptV")
                          for j in range(4):
                              mt = 4 * g + j
                              nc.tensor.transpose(
                                  ptV[:, j, :],
                                  vsb[:, mt * 128:(mt + 1) * 128], ident_bf[:])
                          dstv = vap[p][:, 4 * g:4 * g + 4, :].rearrange(
                              "p m (h d) -> p m h d", h=2, d=65)[:, :, :, 0:64]
                          nc.scalar.copy(
                              dstv, ptV.rearrange("p m (h d) -> p m h d",
                                                  h=2, d=64))

          # ---- Stage C: attention per pair (+ interleaved out-proj) ----
          with ExitStack() as s3:
              s_ps = s3.enter_context(tc.tile_pool(name="sps", bufs=2,
                                                   space="PSUM"))
              a_ps = s3.enter_context(tc.tile_pool(name="aps", bufs=1,
                                                   space="PSUM"))
              po_ps = s3.enter_context(tc.tile_pool(name="pops", bufs=2,
                                                    space="PSUM"))
              p_pool = s3.enter_context(tc.tile_pool(name="pp", bufs=2))
              nrm_p = s3.enter_context(tc.tile_pool(name="nrm", bufs=2))
              ye_p = s3.enter_context(tc.tile_pool(name="ye", bufs=3))

              scale = float(DH) ** -0.5

              def boundary_pieces(pp, pqc, pacc, pP, tail=False):
                  """Previous-qchunk epilogue as (target_mt, fn) pieces.
                  PE work is chopped into <=~0.45us pieces scheduled at
                  the mt where their inputs are ready, so the in-order
                  PE stream never blocks on the slow DVE reciprocal
                  (four [1,256] chunks, ~1.7us each). tail=True is the
                  final drain: latency-optimized (D row first on DVE,
                  acc evac + ye evacs on the now-idle ACT, fp32 bcast
                  quarters gated per recip chunk)."""
                  qsl_n = slice(pqc * W, (pqc + 1) * W)
                  ou16 = nrm_p.tile([64, 2, W], BF16, tag="ou16")
                  ds = nrm_p.tile([1, 2 * W], F32, tag="ds")
                  rds = nrm_p.tile([1, 2 * W], F32, tag="rds")
                  rds16 = nrm_p.tile([1, 2 * W], BF16, tag="rds16")
                  rbp = [None, None]

                  def pv_flush():
                      for hh in range(2):
                          nc.tensor.matmul(
                              pacc[0:65, hh, :],
                              vap[pp][:, NT - 1, hh * 65:(hh + 1) * 65],
                              pP[:, NT - 1, hh, :], start=False, stop=True)
                      if tail:
                          nc.scalar.copy(ou16[:], pacc[0:64, :, :])
                          nc.vector.tensor_copy(
                              ds[:], pacc[64:65, :, :].rearrange(
                                  "p a b -> p (a b)"))
                          for k in range(4):
                              ck = slice(k * 256, (k + 1) * 256)
                              nc.vector.reciprocal(rds[:, ck], ds[:, ck])
                          return
                      # evacuate numerators + D row (frees the acc psum
                      # banks), then 1/D in 4 chunks.
                      nc.vector.tensor_copy(ou16[:], pacc[0:64, :, :])
                      nc.vector.tensor_copy(
                          ds[:], pacc[64:65, :, :].rearrange(
                              "p a b -> p (a b)"))
                      for k in range(4):
                          ck = slice(k * 256, (k + 1) * 256)
                          nc.vector.reciprocal(rds[:, ck], ds[:, ck])

                  def bcast32(k):
                      def f():
                          hh, half = k // 2, k % 2
                          if rbp[hh] is None:
                              rbp[hh] = po_ps.tile([128, W], F32, tag="po",
                                                   name=f"rbp{hh}")
                          cs = slice(half * 256, (half + 1) * 256)
                          nc.tensor.matmul(rbp[hh][0:64, cs], ones_f32[:, :],
                                           rds[:, k * 256:(k + 1) * 256],
                                           start=True, stop=True)
                      return f

                  def nmul():
                      for hh in range(2):
                          nc.vector.tensor_tensor(
                              outn[pp][hh * 64:hh * 64 + 64, qsl_n],
                              ou16[:, hh, :], rbp[hh][0:64, :], op=OP.mult)

                  def oproj(mi):
                      def f():
                          po = po_ps.tile([128, W], F32, tag="po",
                                          name=f"po{mi}")
                          for pr in range(2):
                              nc.tensor.matmul(
                                  po[:],
                                  wout_sb[:, pr, mi * 128:(mi + 1) * 128],
                                  outn[pr][:, qsl_n], start=(pr == 0),
                                  stop=(pr == 1))
                          ye = ye_p.tile([128, W], BF16, tag="ye")
                          if tail:
                              nc.scalar.copy(ye[:], po[:])
                          else:
                              nc.vector.tensor_copy(ye[:], po[:])
                          nc.sync.dma_start(
                              y[mi * 128:(mi + 1) * 128, qsl_n], ye[:])
                      return f

                  if tail:
                      pieces = [(1, pv_flush)]
                      pieces += [(2 + k, bcast32(k)) for k in range(4)]
                      pieces.append((6, nmul))
                      pieces += [(7 + mi, oproj(mi)) for mi in range(4)]
                      return pieces

                  pieces = [(1, pv_flush), (5, bcast32(0)), (6, bcast32(1)),
                            (8, bcast32(2)), (10, bcast32(3)), (11, nmul)]
                  if pp == 1:
                      pieces += [(12 + mi, oproj(mi)) for mi in range(4)]
                  return pieces

              pend = []
              for p in range(2):
                  for qc in range(N // W):
                      qsl = slice(qc * W, (qc + 1) * W)
                      P = p_pool.tile([128, NT, 2, W], BF16, name="P",
                                      tag="P")
                      acc = None
                      pv_done = 0    # PVs deferred past the acc-release
                      for mt in range(NT):
                          s3t = s_ps.tile([128, 2, W], F32, tag="s3")
                          for hh in range(2):
                              hsl = slice(hh * 64, hh * 64 + 64)
                              nc.tensor.matmul(
                                  s3t[:, hh, :],
                                  ks[p][hsl, mt * 128:(mt + 1) * 128],
                                  qs[p][hsl, qsl],
                                  start=True, stop=True)
                          nc.scalar.activation(
                              P[:, mt, :, :], s3t[:], ACT.Exp, scale=scale)
                          while pend and pend[0][0] <= mt:
                              pend.pop(0)[1]()
                          if mt == 1:
                              acc = a_ps.tile([128, 2, W], F32, name="acc",
                                              tag="acc")
                          # PV stream: none at mts 1-3 (the prev qchunk's
                          # acc evac hasn't freed the banks yet; a parked
                          # PV would block the in-order PE), then catch
                          # up two per mt.
                          if mt >= 4:
                              tgt = min(mt, NT - 1)
                              while pv_done < tgt:
                                  j = pv_done
                                  for hh in range(2):
                                      nc.tensor.matmul(
                                          acc[0:65, hh, :],
                                          vap[p][:, j,
                                                 hh * 65:(hh + 1) * 65],
                                          P[:, j, hh, :],
                                          start=(j == 0), stop=False)
                                  pv_done += 1
                                  if pv_done >= min(2 * (mt - 3), tgt):
                                      break
                      while pend:
                          pend.pop(0)[1]()
                      pend = boundary_pieces(p, qc, acc, P,
                                             tail=(p == 1 and
                                                   qc == N // W - 1))
              while pend:
                  pend.pop(0)[1]()

    _split_multiwait(nc)
    return nc


def _host_prep(x, ln_gamma, ln_beta, w_qkv, w_out):
    """Build the 8 per-core input maps."""
    import ml_dtypes
    f32 = np.float32
    bf16 = ml_dtypes.bfloat16
    pos = np.arange(N, dtype=f32)[:, None]
    idx = np.arange(DH, dtype=f32)[None, :]
    angle = pos / (f32(10000.0) ** (idx / f32(DH)))       # [N, DH]
    cos2 = np.ascontiguousarray(np.tile(np.cos(angle).T, (2, 1))).astype(bf16)
    sin2 = np.ascontiguousarray(np.tile(np.sin(angle).T, (2, 1))).astype(bf16)
    ident = np.eye(128, dtype=f32)
    roll64 = np.zeros((64, 64), f32)
    for p in range(64):
        roll64[(p - 1) % 64, p] = 1.0     # lhsT[src, dst]: dst p <- src p-1
    r2 = np.zeros((128, 128), f32)
    r2[0:64, 0:64] = roll64
    r2[64:128, 64:128] = roll64
    r2 = r2.astype(bf16)

    wg = (w_qkv * ln_gamma[:, None]).astype(f32)          # [512, 1536]
    beta_row = (ln_beta @ w_qkv).astype(f32)              # [1536]

    def head_block(a, sec, h):    # sec 0=q 1=k 2=v, global head h
        return a[..., sec * 512 + h * DH: sec * 512 + (h + 1) * DH]

    in_maps = []
    for c in range(8):
        bi, hg = c // 2, c % 2
        hs = [4 * hg + i for i in range(HPC)]
        mts, bcols = [], []
        # M-tile order: k01 q01 v01 k23 q23 v23
        for pr in range(2):
            for sec in (1, 0, 2):
                mts.append(np.concatenate(
                    [head_block(wg, sec, hs[2 * pr]),
                     head_block(wg, sec, hs[2 * pr + 1])], axis=1))
                bcols.append(np.concatenate(
                    [head_block(beta_row, sec, hs[2 * pr]),
                     head_block(beta_row, sec, hs[2 * pr + 1])]))
        wqkv_c = np.ascontiguousarray(
            np.concatenate(mts, axis=1)).astype(bf16)     # [512, 768]
        beta_c = np.stack(bcols, axis=1).astype(f32)      # [128, 6]
        wout_c = np.ascontiguousarray(
            w_out[hg * 256:(hg + 1) * 256, :]).astype(bf16)
        in_maps.append({
            "x_nat": np.ascontiguousarray(x[bi]).astype(bf16),
            "wqkv": wqkv_c,
            "beta_mt": beta_c,
            "r2": r2,
            "wout": wout_c,
            "cos2": cos2,
            "sin2": sin2,
            "ident": ident,
        })
    return in_maps


_NC = None


def kernel(x, ln_gamma, ln_beta, w_qkv, w_out, b_out, **run_kwargs):
    global _NC
    x = np.asarray(x, dtype=np.float32)
    assert x.shape == (B, N, D), x.shape
    if _NC is None:
        _NC = build_nc()
    in_maps = _host_prep(np.asarray(x), np.asarray(ln_gamma),
                         np.asarray(ln_beta), np.asarray(w_qkv),
                         np.asarray(w_out))
    res = run_bass_kernel_spmd(_NC, in_maps, core_ids=list(range(8)), **run_kwargs)
    out = np.empty((B, N, D), dtype=np.float32)
    for bi in range(B):
        part = (res.results[2 * bi]["y"].astype(np.float32)
                + res.results[2 * bi + 1]["y"].astype(np.float32))
        out[bi] = part.T + np.asarray(b_out, dtype=np.float32)
    kernel.last_results = res
    return out


# revision 35
# speedup vs baseline: 1.1418x; 1.0287x over previous
"""Fused LN + QKV + RoPE + attention + out-proj Trainium2 kernel, v4.

Shapes (hardcoded from the problem spec):
  x [4, 2048, 512] fp32, w_qkv [512, 1536], w_out [512, 512],
  ln_gamma/ln_beta/b_out [512]. 8 heads of 64. Output [4, 2048, 512].

Sharding: 8 cores = 4 batches x 2 head-groups (4 heads each). Each core
computes a w_out row-split partial output for its batch; the host sums
the two partials per batch and adds b_out.

Design notes (ACT-exp is the roofline: 16.8M exp/core ~= 110us min):
 - LN is two-pass: per-tile sum (DVE reduce) and sum-of-squares, then
   ONE batched sqrt + reciprocal, then per-tile xn (bf16 so the PE
   transpose runs at 1 cycle/row).
 - QKV: 6 M-tiles (k/q/v per pair); RoPE's roll computed by a
   block-diagonal permutation matmul on (q + beta); combine split
   across GpSimd (t*cos) and DVE (pr*sin, final bf16 add in 2x mode).
 - Attention per head-pair, per-mt software pipeline: QK row-tiled
   2 heads concurrently in the 128x128 PE array into one [128,2,512]
   PSUM slab (2 rotating slabs), ONE exp per mt covering both heads so
   ACT runs back-to-back; PV (ones-augmented V, M=65, row 64 = softmax
   denominator) trails exp by one mt. PE order QK(mt+1) before PV(mt)
   so the in-order PE never blocks the exp chain.
 - Normalize: reciprocal_approx_fast (~5x faster than DVE RECIPROCAL)
   on both heads' D rows, fp32 ones-matmul broadcast into rows 64:128
   of the acc's own psum banks, one fused scalar_tensor_tensor
   (acc * 1/D) per head. Emitted at the START of the next qchunk.
 - Out-proj interleaved: emitted per-qchunk during pair 1's attention
   (needs both pairs' outn), evacuated on DVE (ACT is exp-saturated).
 - PSUM: s3 slabs 2x2 + acc 2 + out-proj 2 = 8 banks exactly.
Matmul operands bf16 (fp32 PSUM accumulation); LN/softmax math fp32.
"""

import numpy as np

import concourse.bass as bass
import concourse.tile as tile
from concourse import mybir
from concourse.bass_utils import run_bass_kernel_spmd

F32 = mybir.dt.float32
BF16 = mybir.dt.bfloat16
AX = mybir.AxisListType
OP = mybir.AluOpType
ACT = mybir.ActivationFunctionType

B, N, D = 4, 2048, 512
HEADS, DH = 8, 64
HPC = 4            # heads per core
EPS = 1e-5
NT = N // 128      # 16 token tiles
KT = D // 128      # 4 feature tiles
W = 512            # attention query-chunk width


def _split_multiwait(nc):
    """Insert NoOps so no instruction carries more than one sem wait.

    The pinned walrus rejects >1 sync wait per instruction
    (setupSyncWait "Too many sync wait commands"). Waits are a
    conjunction, so hoisting all but the last onto same-engine NoOps
    immediately before the instruction is equivalent.
    """
    ctr = 0
    for fn in nc.m.functions:
        for blk in fn.blocks:
            insts = blk.instructions
            idx = 0
            while idx < len(insts):
                inst = insts[idx]
                si = inst.sync_info
                if si is not None and len(si.on_wait) > 1:
                    waits = list(si.on_wait)
                    for w in waits[:-1]:
                        nop = mybir.InstNoOp(name=f"SWNOP-{ctr}", ins=[], outs=[])
                        ctr += 1
                        nop.engine = inst.engine
                        nop.sync_info = mybir.SyncInfo(on_wait=[w], on_update=[])
                        insts.insert(idx, nop)
                        idx += 1
                    inst.sync_info = mybir.SyncInfo(
                        on_wait=[waits[-1]], on_update=list(si.on_update)
                    )
                idx += 1


def build_nc(loops=1):
    from contextlib import ExitStack

    nc = bass.Bass("TRN2", target_bir_lowering=False, num_devices=8)

    x_nat = nc.dram_tensor("x_nat", [N, D], BF16, kind="ExternalInput")
    # gamma-folded QKV weights bf16, M-tile order k01 q01 v01 k23 q23 v23
    wqkv = nc.dram_tensor("wqkv", [D, 768], BF16, kind="ExternalInput")
    beta_mt = nc.dram_tensor("beta_mt", [128, 6], F32, kind="ExternalInput")
    r2 = nc.dram_tensor("r2", [128, 128], BF16, kind="ExternalInput")
    wout = nc.dram_tensor("wout", [HPC * DH, D], BF16, kind="ExternalInput")
    cos2 = nc.dram_tensor("cos2", [128, N], BF16, kind="ExternalInput")
    sin2 = nc.dram_tensor("sin2", [128, N], BF16, kind="ExternalInput")
    ident = nc.dram_tensor("ident", [128, 128], F32, kind="ExternalInput")
    y = nc.dram_tensor("y", [D, N], BF16, kind="ExternalOutput")

    with tile.TileContext(nc) as tc:
      for _loop in range(loops):
        with ExitStack() as ctx:
          const = ctx.enter_context(tc.tile_pool(name="const", bufs=1))
          qk_pool = ctx.enter_context(tc.tile_pool(name="qk", bufs=1))
          va_pool = ctx.enter_context(tc.tile_pool(name="va", bufs=1))
          outn_pool = ctx.enter_context(tc.tile_pool(name="outn", bufs=1))

          ident_sb = const.tile([128, 128], F32)
          nc.gpsimd.dma_start(ident_sb[:], ident[:, :])
          ident_bf = const.tile([128, 128], BF16)
          nc.vector.tensor_copy(ident_bf[:], ident_sb[:])
          r2_sb = const.tile([128, 128], BF16)
          nc.gpsimd.dma_start(r2_sb[:], r2[:, :])
          eps_sb = const.tile([128, 1], F32)
          nc.vector.memset(eps_sb[:], EPS)
          ones_f32 = const.tile([1, 64], F32)
          nc.vector.memset(ones_f32[:], 1.0)
          ones_bf = const.tile([1, 64], BF16)
          nc.vector.memset(ones_bf[:], 1.0)
          ones_bf8 = const.tile([8, 64], BF16)
          nc.vector.memset(ones_bf8[:], 1.0)
          beta_sb = const.tile([128, 6], F32)
          nc.gpsimd.dma_start(beta_sb[:], beta_mt[:, :])
          wq_sb = const.tile([128, KT, 768], BF16, name="wq")
          for kt in range(KT):
              nc.gpsimd.dma_start(wq_sb[:, kt, :],
                                  wqkv[kt * 128:(kt + 1) * 128, :])
          wout_sb = const.tile([128, 2, D], BF16, name="wout")
          for p in range(2):
              nc.scalar.dma_start(wout_sb[:, p, :],
                                  wout[p * 128:(p + 1) * 128, :])
          cos_sb = const.tile([128, N], BF16, name="cos")
          nc.scalar.dma_start(cos_sb[:], cos2[:, :])
          sin_sb = const.tile([128, N], BF16, name="sin")
          nc.scalar.dma_start(sin_sb[:], sin2[:, :])

          # q/k rope'd feature-major per pair [128, N]; vap per pair holds
          # both heads' V ktok-major with ones columns at 64 and 129.
          qs = [qk_pool.tile([128, N], BF16, name=f"qs{p}", tag=f"qs{p}")
                for p in range(2)]
          ks = [qk_pool.tile([128, N], BF16, name=f"ks{p}", tag=f"ks{p}")
                for p in range(2)]
          vap = [va_pool.tile([128, NT, 130], BF16, name=f"vap{p}",
                              tag=f"vap{p}") for p in range(2)]
          for p in range(2):
              nc.vector.memset(vap[p][:], 1.0)
          outn = [outn_pool.tile([128, N], BF16, name=f"on{p}", tag=f"on{p}")
                  for p in range(2)]

          # ---- Stage A: LayerNorm (two-pass) + PE transpose ----
          with ExitStack() as s1:
              x_p = s1.enter_context(tc.tile_pool(name="x", bufs=1))
              st_p = s1.enter_context(tc.tile_pool(name="st", bufs=1))
              xn_p = s1.enter_context(tc.tile_pool(name="xn", bufs=3))
              scr_p = s1.enter_context(tc.tile_pool(name="scr", bufs=2))
              xnT_p = s1.enter_context(tc.tile_pool(name="xnT", bufs=1))
              ptA_ps = s1.enter_context(tc.tile_pool(name="ptA", bufs=1,
                                                     space="PSUM"))

              xts = x_p.tile([128, NT, D], BF16, name="xts")
              muvar = st_p.tile([128, NT, 2], F32, name="muvar")
              # x loads split across the sync and gpsimd DMA queues so
              # the tiles land twice as fast.
              for tt in range(NT):
                  eng = nc.sync if tt % 2 == 0 else nc.gpsimd
                  eng.dma_start(xts[:, tt, :],
                                x_nat[tt * 128:(tt + 1) * 128, :])
              mu_all = muvar[:, :, 0:1].rearrange("p a b -> p (a b)")
              var_all = muvar[:, :, 1:2].rearrange("p a b -> p (a b)")
              sd_all = st_p.tile([128, NT], F32, name="sd_all")
              rs_all = st_p.tile([128, NT], F32, name="rs_all")
              bias2 = st_p.tile([128, NT], F32, name="bias2")
              xnT = xnT_p.tile([128, KT, N], BF16, name="xnT")

              # batches of 4 tiles: stats -> batch sqrt/recip -> xn +
              # transpose + evac, so pass 2 of batch b overlaps pass 1
              # of batch b+1 instead of waiting for all 16 tiles.
              for b in range(NT // 4):
                  bs = slice(4 * b, 4 * b + 4)
                  for j in range(4):
                      tt = 4 * b + j
                      bn6 = scr_p.tile([128, 6], F32, tag="bn6")
                      nc.vector.bn_stats(bn6[:], xts[:, tt, :])
                      nc.vector.bn_aggr(muvar[:, tt, :], bn6[:])
                  nc.scalar.activation(sd_all[:, bs], var_all[:, bs],
                                       ACT.Sqrt, bias=eps_sb[:])
                  nc.vector.reciprocal(rs_all[:, bs], sd_all[:, bs])
                  nc.vector.scalar_tensor_tensor(
                      bias2[:, bs], mu_all[:, bs], -1.0, rs_all[:, bs],
                      op0=OP.mult, op1=OP.mult)
                  for j in range(4):
                      tt = 4 * b + j
                      xn = xn_p.tile([128, D], BF16, tag="xn")
                      if tt % 2 == 0:
                          nc.scalar.activation(
                              xn[:], xts[:, tt, :], ACT.Identity,
                              bias=bias2[:, tt:tt + 1],
                              scale=rs_all[:, tt:tt + 1])
                      else:
                          nc.vector.tensor_scalar(
                              xn[:], xts[:, tt, :], muvar[:, tt, 0:1],
                              rs_all[:, tt:tt + 1], op0=OP.subtract,
                              op1=OP.mult)
                      pt = ptA_ps.tile([128, KT, 128], BF16, tag="pt")
                      for ft in range(KT):
                          nc.tensor.transpose(
                              pt[:, ft, :], xn[:, ft * 128:(ft + 1) * 128],
                              ident_bf[:])
                      if tt % 2 == 0:
                          nc.vector.tensor_copy(
                              xnT[:, :, tt * 128:(tt + 1) * 128], pt[:])
                      else:
                          nc.scalar.copy(
                              xnT[:, :, tt * 128:(tt + 1) * 128], pt[:])

              # ---- Stage B: QKV + RoPE per pair ----
              with ExitStack() as s2:
                  pq_ps = s2.enter_context(tc.tile_pool(name="pq", bufs=2,
                                                        space="PSUM"))
                  pr_ps = s2.enter_context(tc.tile_pool(name="pr", bufs=1,
                                                        space="PSUM"))
                  ptV_ps = s2.enter_context(tc.tile_pool(name="ptV", bufs=1,
                                                         space="PSUM"))
                  t_p = s2.enter_context(tc.tile_pool(name="t", bufs=3))
                  t1_p = s2.enter_context(tc.tile_pool(name="t1", bufs=2))
                  vsb_p = s2.enter_context(tc.tile_pool(name="vsb", bufs=2))

                  def bm(m):
                      return beta_sb[:, m:m + 1]

                  def qkv_mm(psum_ap, m, half):
                      ms = slice(m * 128, (m + 1) * 128)
                      for nn in range(2):
                          cs = slice(half * 1024 + nn * 512,
                                     half * 1024 + (nn + 1) * 512)
                          for kt in range(KT):
                              nc.tensor.matmul(
                                  psum_ap[:, nn * 512:(nn + 1) * 512],
                                  wq_sb[:, kt, ms], xnT[:, kt, cs],
                                  start=(kt == 0), stop=(kt == KT - 1))

                  for p in range(2):
                      vsb = vsb_p.tile([128, N], BF16, tag=f"vsb{p}")
                      for half in range(2):
                          hs = slice(half * 1024, (half + 1) * 1024)
                          for sec, dst in ((0, ks[p]), (1, qs[p])):
                              m = 3 * p + sec
                              pq = pq_ps.tile([128, 1024], F32, tag="pq")
                              qkv_mm(pq, m, half)
                              # t = raw + beta (bf16), roll via perm matmul
                              t = t_p.tile([128, 1024], BF16, tag="t")
                              nc.scalar.add(t[:], pq[:], bm(m))
                              pr = pr_ps.tile([128, 1024], F32, tag="pr")
                              for nn in range(2):
                                  nc.tensor.matmul(
                                      pr[:, nn * 512:(nn + 1) * 512], r2_sb[:],
                                      t[:, nn * 512:(nn + 1) * 512],
                                      start=True, stop=True)
                              # dst = t*cos + roll(t)*sin; t*cos
                              # alternates GpSimd/DVE to balance load
                              t1 = t1_p.tile([128, 1024], BF16, tag="t1")
                              t1e = nc.gpsimd if (half + sec) % 2 else \
                                  nc.vector
                              t1e.tensor_tensor(
                                  t1[:], t[:], cos_sb[:, hs], op=OP.mult)
                              nc.vector.scalar_tensor_tensor(
                                  dst[:, hs], pr[:], 0.0, sin_sb[:, hs],
                                  op0=OP.add, op1=OP.mult)
                              nc.vector.tensor_tensor(
                                  dst[:, hs], dst[:, hs], t1[:], op=OP.add)
                          # v
                          m = 3 * p + 2
                          pv = pq_ps.tile([128, 1024], F32, tag="pq")
                          qkv_mm(pv, m, half)
                          nc.vector.tensor_scalar_add(vsb[:, hs], pv[:], bm(m))
                      # transpose v to ktok-major, 4 tiles per psum bank,
                      # one fused strided evac per group into the paired
                      # [v_h0|1|v_h1|1] layout.
                      for g in range(NT // 4):
                          ptV = ptV_ps.tile([128, 4, 128], BF16, tag="ptV")
                          for j in range(4):
                              mt = 4 * g + j
                              nc.tensor.transpose(
                                  ptV[:, j, :],
                                  vsb[:, mt * 128:(mt + 1) * 128], ident_bf[:])
                          dstv = vap[p][:, 4 * g:4 * g + 4, :].rearrange(
                              "p m (h d) -> p m h d", h=2, d=65)[:, :, :, 0:64]
                          nc.scalar.copy(
                              dstv, ptV.rearrange("p m (h d) -> p m h d",
                                                  h=2, d=64))

          # ---- Stage C: attention per pair (+ interleaved out-proj) ----
          with ExitStack() as s3:
              s_ps = s3.enter_context(tc.tile_pool(name="sps", bufs=2,
                                                   space="PSUM"))
              a_ps = s3.enter_context(tc.tile_pool(name="aps", bufs=1,
                                                   space="PSUM"))
              po_ps = s3.enter_context(tc.tile_pool(name="pops", bufs=2,
                                                    space="PSUM"))
              p_pool = s3.enter_context(tc.tile_pool(name="pp", bufs=2))
              nrm_p = s3.enter_context(tc.tile_pool(name="nrm", bufs=2))
              ye_p = s3.enter_context(tc.tile_pool(name="ye", bufs=3))

              scale = float(DH) ** -0.5

              def boundary_pieces(pp, pqc, pacc, pP, tail=False):
                  """Previous-qchunk epilogue as (target_mt, fn) pieces.
                  PE work is chopped into <=~0.45us pieces scheduled at
                  the mt where their inputs are ready, so the in-order
                  PE stream never blocks on the slow DVE reciprocal
                  (four [1,256] chunks, ~1.7us each). tail=True is the
                  final drain: latency-optimized (D row first on DVE,
                  acc evac + ye evacs on the now-idle ACT, fp32 bcast
                  quarters gated per recip chunk)."""
                  qsl_n = slice(pqc * W, (pqc + 1) * W)
                  ou16 = nrm_p.tile([64, 2, W], BF16, tag="ou16")
                  # D row lives on one partition; 1/D at 5.7ns/elem on
                  # DVE would serialize ~6us. Instead: spread D across
                  # 128 partitions via PE transposes (column 0 of each
                  # [128,128] block), reciprocal on [128,4] (cheap),
                  # transpose back, and broadcast the bf16 result.
                  ds = nrm_p.tile([128, 2 * W], F32, tag="ds")
                  rts = nrm_p.tile([128, 136], F32, tag="rts")
                  rr16 = nrm_p.tile([1, 2 * W], BF16, tag="rr16")
                  hold = {}

                  def pv_flush():
                      for hh in range(2):
                          nc.tensor.matmul(
                              pacc[0:65, hh, :],
                              vap[pp][:, NT - 1, hh * 65:(hh + 1) * 65],
                              pP[:, NT - 1, hh, :], start=False, stop=True)
                      if tail:
                          nc.scalar.copy(ou16[:], pacc[0:64, :, :])
                      else:
                          nc.vector.tensor_copy(ou16[:], pacc[0:64, :, :])
                      nc.vector.tensor_copy(
                          ds[0:1, :], pacc[64:65, :, :].rearrange(
                              "p a b -> p (a b)"))

                  def tblk(g):      # transpose 4 D segments into psum
                      def f():
                          pt = po_ps.tile([128, W], F32, tag="po",
                                          name=f"pt{g}")
                          hold[f"pt{g}"] = pt
                          for j in range(4):
                              k = 4 * g + j
                              nc.tensor.transpose(
                                  pt[:, j * 128:(j + 1) * 128],
                                  ds[:, k * 128:(k + 1) * 128], ident_sb[:])
                          cols = pt[:].rearrange("p (j c) -> p j c",
                                                 j=4, c=128)[:, :, 0]
                          nc.vector.reciprocal(rts[:, g * 4:g * 4 + 4],
                                               cols)
                      return f

                  def tback(g):
                      # shifted transposes: input column-offset j puts
                      # 1/D segment j on psum ROW 0, evacuated into one
                      # [1, 2W] bf16 row for the partition-0 broadcast.
                      def f():
                          pt = po_ps.tile([128, W], F32, tag="po",
                                          name=f"ptb{g}")
                          for c in range(4):
                              j = 4 * g + c
                              nc.tensor.transpose(
                                  pt[:, c * 128:(c + 1) * 128],
                                  rts[:, j:j + 128], ident_sb[:])
                          nc.vector.tensor_copy(
                              rr16[0:1, g * W:(g + 1) * W], pt[0:1, :])
                      return f

                  def bcast(hh):
                      def f():
                          rbp = po_ps.tile([128, W], F32, tag="po",
                                           name=f"rbp{hh}")
                          hold[f"rbp{hh}"] = rbp
                          for c in range(4):
                              j = hh * 4 + c
                              nc.tensor.matmul(
                                  rbp[0:64, c * 128:(c + 1) * 128],
                                  ones_bf[:, :],
                                  rr16[0:1, j * 128:(j + 1) * 128],
                                  start=True, stop=True)
                      return f

                  def nmul():
                      for hh in range(2):
                          nc.vector.tensor_tensor(
                              outn[pp][hh * 64:hh * 64 + 64, qsl_n],
                              ou16[:, hh, :], hold[f"rbp{hh}"][0:64, :],
                              op=OP.mult)

                  def oproj(mi):
                      def f():
                          po = po_ps.tile([128, W], F32, tag="po",
                                          name=f"po{mi}")
                          for pr in range(2):
                              nc.tensor.matmul(
                                  po[:],
                                  wout_sb[:, pr, mi * 128:(mi + 1) * 128],
                                  outn[pr][:, qsl_n], start=(pr == 0),
                                  stop=(pr == 1))
                          ye = ye_p.tile([128, W], BF16, tag="ye")
                          if tail:
                              nc.scalar.copy(ye[:], po[:])
                          else:
                              nc.vector.tensor_copy(ye[:], po[:])
                          nc.sync.dma_start(
                              y[mi * 128:(mi + 1) * 128, qsl_n], ye[:])
                      return f

                  base = [(1, pv_flush), (3, tblk(0)), (4, tblk(1)),
                          (5, tback(0)), (6, tback(1)), (7, bcast(0)),
                          (8, bcast(1)), (9, nmul)]
                  if tail:
                      return base + [(10 + mi, oproj(mi)) for mi in range(4)]
                  if pp == 1:
                      base += [(10 + mi, oproj(mi)) for mi in range(4)]
                  return base

              pend = []
              for p in range(2):
                  for qc in range(N // W):
                      qsl = slice(qc * W, (qc + 1) * W)
                      P = p_pool.tile([128, NT, 2, W], BF16, name="P",
                                      tag="P")
                      acc = None
                      pv_done = 0    # PVs deferred past the acc-release
                      for mt in range(NT):
                          s3t = s_ps.tile([128, 2, W], F32, tag="s3")
                          for hh in range(2):
                              hsl = slice(hh * 64, hh * 64 + 64)
                              nc.tensor.matmul(
                                  s3t[:, hh, :],
                                  ks[p][hsl, mt * 128:(mt + 1) * 128],
                                  qs[p][hsl, qsl],
                                  start=True, stop=True)
                          nc.scalar.activation(
                              P[:, mt, :, :], s3t[:], ACT.Exp, scale=scale)
                          while pend and pend[0][0] <= mt:
                              pend.pop(0)[1]()
                          if mt == 1:
                              acc = a_ps.tile([128, 2, W], F32, name="acc",
                                              tag="acc")
                          # PV stream: none at mts 1-3 (the prev qchunk's
                          # acc evac hasn't freed the banks yet; a parked
                          # PV would block the in-order PE), then catch
                          # up two per mt.
                          if mt >= 4:
                              tgt = min(mt, NT - 1)
                              while pv_done < tgt:
                                  j = pv_done
                                  for hh in range(2):
                                      nc.tensor.matmul(
                                          acc[0:65, hh, :],
                                          vap[p][:, j,
                                                 hh * 65:(hh + 1) * 65],
                                          P[:, j, hh, :],
                                          start=(j == 0), stop=False)
                                  pv_done += 1
                                  if pv_done >= min(2 * (mt - 3), tgt):
                                      break
                      while pend:
                          pend.pop(0)[1]()
                      pend = boundary_pieces(p, qc, acc, P,
                                             tail=(p == 1 and
                                                   qc == N // W - 1))
              while pend:
                  pend.pop(0)[1]()

    _split_multiwait(nc)
    return nc


def _host_prep(x, ln_gamma, ln_beta, w_qkv, w_out):
    """Build the 8 per-core input maps."""
    import ml_dtypes
    f32 = np.float32
    bf16 = ml_dtypes.bfloat16
    pos = np.arange(N, dtype=f32)[:, None]
    idx = np.arange(DH, dtype=f32)[None, :]
    angle = pos / (f32(10000.0) ** (idx / f32(DH)))       # [N, DH]
    cos2 = np.ascontiguousarray(np.tile(np.cos(angle).T, (2, 1))).astype(bf16)
    sin2 = np.ascontiguousarray(np.tile(np.sin(angle).T, (2, 1))).astype(bf16)
    ident = np.eye(128, dtype=f32)
    roll64 = np.zeros((64, 64), f32)
    for p in range(64):
        roll64[(p - 1) % 64, p] = 1.0     # lhsT[src, dst]: dst p <- src p-1
    r2 = np.zeros((128, 128), f32)
    r2[0:64, 0:64] = roll64
    r2[64:128, 64:128] = roll64
    r2 = r2.astype(bf16)

    wg = (w_qkv * ln_gamma[:, None]).astype(f32)          # [512, 1536]
    beta_row = (ln_beta @ w_qkv).astype(f32)              # [1536]

    def head_block(a, sec, h):    # sec 0=q 1=k 2=v, global head h
        return a[..., sec * 512 + h * DH: sec * 512 + (h + 1) * DH]

    in_maps = []
    for c in range(8):
        bi, hg = c // 2, c % 2
        hs = [4 * hg + i for i in range(HPC)]
        mts, bcols = [], []
        # M-tile order: k01 q01 v01 k23 q23 v23
        for pr in range(2):
            for sec in (1, 0, 2):
                mts.append(np.concatenate(
                    [head_block(wg, sec, hs[2 * pr]),
                     head_block(wg, sec, hs[2 * pr + 1])], axis=1))
                bcols.append(np.concatenate(
                    [head_block(beta_row, sec, hs[2 * pr]),
                     head_block(beta_row, sec, hs[2 * pr + 1])]))
        wqkv_c = np.ascontiguousarray(
            np.concatenate(mts, axis=1)).astype(bf16)     # [512, 768]
        beta_c = np.stack(bcols, axis=1).astype(f32)      # [128, 6]
        wout_c = np.ascontiguousarray(
            w_out[hg * 256:(hg + 1) * 256, :]).astype(bf16)
        in_maps.append({
            "x_nat": np.ascontiguousarray(x[bi]).astype(bf16),
            "wqkv": wqkv_c,
            "beta_mt": beta_c,
            "r2": r2,
            "wout": wout_c,
            "cos2": cos2,
            "sin2": sin2,
            "ident": ident,
        })
    return in_maps


_NC = None


def kernel(x, ln_gamma, ln_beta, w_qkv, w_out, b_out, **run_kwargs):
    global _NC
    x = np.asarray(x, dtype=np.float32)
    assert x.shape == (B, N, D), x.shape
    if _NC is None:
        _NC = build_nc()
    in_maps = _host_prep(np.asarray(x), np.asarray(ln_gamma),
                         np.asarray(ln_beta), np.asarray(w_qkv),
                         np.asarray(w_out))
    res = run_bass_kernel_spmd(_NC, in_maps, core_ids=list(range(8)), **run_kwargs)
    out = np.empty((B, N, D), dtype=np.float32)
    for bi in range(B):
        part = (res.results[2 * bi]["y"].astype(np.float32)
                + res.results[2 * bi + 1]["y"].astype(np.float32))
        out[bi] = part.T + np.asarray(b_out, dtype=np.float32)
    kernel.last_results = res
    return out


# revision 40
# speedup vs baseline: 1.1838x; 1.0368x over previous
"""Fused LN + QKV + RoPE + attention + out-proj Trainium2 kernel, v4.

Shapes (hardcoded from the problem spec):
  x [4, 2048, 512] fp32, w_qkv [512, 1536], w_out [512, 512],
  ln_gamma/ln_beta/b_out [512]. 8 heads of 64. Output [4, 2048, 512].

Sharding: 8 cores = 4 batches x 2 head-groups (4 heads each). Each core
computes a w_out row-split partial output for its batch; the host sums
the two partials per batch and adds b_out.

Design notes (ACT-exp is the roofline: 16.8M exp/core ~= 110us min):
 - LN is two-pass: per-tile sum (DVE reduce) and sum-of-squares, then
   ONE batched sqrt + reciprocal, then per-tile xn (bf16 so the PE
   transpose runs at 1 cycle/row).
 - QKV: 6 M-tiles (k/q/v per pair); RoPE's roll computed by a
   block-diagonal permutation matmul on (q + beta); combine split
   across GpSimd (t*cos) and DVE (pr*sin, final bf16 add in 2x mode).
 - Attention per head-pair, per-mt software pipeline: QK row-tiled
   2 heads concurrently in the 128x128 PE array into one [128,2,512]
   PSUM slab (2 rotating slabs), ONE exp per mt covering both heads so
   ACT runs back-to-back; PV (ones-augmented V, M=65, row 64 = softmax
   denominator) trails exp by one mt. PE order QK(mt+1) before PV(mt)
   so the in-order PE never blocks the exp chain.
 - Normalize: reciprocal_approx_fast (~5x faster than DVE RECIPROCAL)
   on both heads' D rows, fp32 ones-matmul broadcast into rows 64:128
   of the acc's own psum banks, one fused scalar_tensor_tensor
   (acc * 1/D) per head. Emitted at the START of the next qchunk.
 - Out-proj interleaved: emitted per-qchunk during pair 1's attention
   (needs both pairs' outn), evacuated on DVE (ACT is exp-saturated).
 - PSUM: s3 slabs 2x2 + acc 2 + out-proj 2 = 8 banks exactly.
Matmul operands bf16 (fp32 PSUM accumulation); LN/softmax math fp32.
"""

import numpy as np

import concourse.bass as bass
import concourse.tile as tile
from concourse import mybir
from concourse.bass_utils import run_bass_kernel_spmd

F32 = mybir.dt.float32
BF16 = mybir.dt.bfloat16
AX = mybir.AxisListType
OP = mybir.AluOpType
ACT = mybir.ActivationFunctionType

B, N, D = 4, 2048, 512
HEADS, DH = 8, 64
HPC = 4            # heads per core
EPS = 1e-5
NT = N // 128      # 16 token tiles
KT = D // 128      # 4 feature tiles
W = 512            # attention query-chunk width


def _split_multiwait(nc):
    """Insert NoOps so no instruction carries more than one sem wait.

    The pinned walrus rejects >1 sync wait per instruction
    (setupSyncWait "Too many sync wait commands"). Waits are a
    conjunction, so hoisting all but the last onto same-engine NoOps
    immediately before the instruction is equivalent.
    """
    ctr = 0
    for fn in nc.m.functions:
        for blk in fn.blocks:
            insts = blk.instructions
            idx = 0
            while idx < len(insts):
                inst = insts[idx]
                si = inst.sync_info
                if si is not None and len(si.on_wait) > 1:
                    waits = list(si.on_wait)
                    for w in waits[:-1]:
                        nop = mybir.InstNoOp(name=f"SWNOP-{ctr}", ins=[], outs=[])
                        ctr += 1
                        nop.engine = inst.engine
                        nop.sync_info = mybir.SyncInfo(on_wait=[w], on_update=[])
                        insts.insert(idx, nop)
                        idx += 1
                    inst.sync_info = mybir.SyncInfo(
                        on_wait=[waits[-1]], on_update=list(si.on_update)
                    )
                idx += 1


def build_nc(loops=1):
    from contextlib import ExitStack

    nc = bass.Bass("TRN2", target_bir_lowering=False, num_devices=8)

    x_nat = nc.dram_tensor("x_nat", [N, D], BF16, kind="ExternalInput")
    # gamma-folded QKV weights bf16, M-tile order k01 q01 v01 k23 q23 v23
    wqkv = nc.dram_tensor("wqkv", [D, 768], BF16, kind="ExternalInput")
    beta_mt = nc.dram_tensor("beta_mt", [128, 6], F32, kind="ExternalInput")
    r2 = nc.dram_tensor("r2", [128, 128], BF16, kind="ExternalInput")
    wout = nc.dram_tensor("wout", [HPC * DH, D], BF16, kind="ExternalInput")
    cos2 = nc.dram_tensor("cos2", [128, N], BF16, kind="ExternalInput")
    sin2 = nc.dram_tensor("sin2", [128, N], BF16, kind="ExternalInput")
    ident = nc.dram_tensor("ident", [128, 128], F32, kind="ExternalInput")
    y = nc.dram_tensor("y", [D, N], BF16, kind="ExternalOutput")

    with tile.TileContext(nc) as tc:
      for _loop in range(loops):
        with ExitStack() as ctx:
          const = ctx.enter_context(tc.tile_pool(name="const", bufs=1))
          qk_pool = ctx.enter_context(tc.tile_pool(name="qk", bufs=1))
          va_pool = ctx.enter_context(tc.tile_pool(name="va", bufs=1))
          outn_pool = ctx.enter_context(tc.tile_pool(name="outn", bufs=1))

          ident_sb = const.tile([128, 128], F32)
          nc.gpsimd.dma_start(ident_sb[:], ident[:, :])
          ident_bf = const.tile([128, 128], BF16)
          nc.vector.tensor_copy(ident_bf[:], ident_sb[:])
          r2_sb = const.tile([128, 128], BF16)
          nc.gpsimd.dma_start(r2_sb[:], r2[:, :])
          eps_sb = const.tile([128, 1], F32)
          nc.vector.memset(eps_sb[:], EPS)
          ones_f32 = const.tile([1, 64], F32)
          nc.vector.memset(ones_f32[:], 1.0)
          ones_bf = const.tile([1, 64], BF16)
          nc.vector.memset(ones_bf[:], 1.0)
          ones_bf8 = const.tile([8, 64], BF16)
          nc.vector.memset(ones_bf8[:], 1.0)
          beta_sb = const.tile([128, 6], F32)
          nc.gpsimd.dma_start(beta_sb[:], beta_mt[:, :])
          wq_sb = const.tile([128, KT, 768], BF16, name="wq")
          for kt in range(KT):
              nc.gpsimd.dma_start(wq_sb[:, kt, :],
                                  wqkv[kt * 128:(kt + 1) * 128, :])
          wout_sb = const.tile([128, 2, D], BF16, name="wout")
          for p in range(2):
              nc.scalar.dma_start(wout_sb[:, p, :],
                                  wout[p * 128:(p + 1) * 128, :])
          cos_sb = const.tile([128, N], BF16, name="cos")
          nc.scalar.dma_start(cos_sb[:], cos2[:, :])
          sin_sb = const.tile([128, N], BF16, name="sin")
          nc.scalar.dma_start(sin_sb[:], sin2[:, :])

          # q/k rope'd feature-major per pair [128, N]; vap per pair holds
          # both heads' V ktok-major with ones columns at 64 and 129.
          qs = [qk_pool.tile([128, N], BF16, name=f"qs{p}", tag=f"qs{p}")
                for p in range(2)]
          ks = [qk_pool.tile([128, N], BF16, name=f"ks{p}", tag=f"ks{p}")
                for p in range(2)]
          vap = [va_pool.tile([128, NT, 130], BF16, name=f"vap{p}",
                              tag=f"vap{p}") for p in range(2)]
          for p in range(2):
              nc.vector.memset(vap[p][:], 1.0)
          outn = [outn_pool.tile([128, N], BF16, name=f"on{p}", tag=f"on{p}")
                  for p in range(2)]

          # ---- Stage A: LayerNorm (two-pass) + PE transpose ----
          with ExitStack() as s1:
              x_p = s1.enter_context(tc.tile_pool(name="x", bufs=1))
              st_p = s1.enter_context(tc.tile_pool(name="st", bufs=1))
              xn_p = s1.enter_context(tc.tile_pool(name="xn", bufs=3))
              scr_p = s1.enter_context(tc.tile_pool(name="scr", bufs=2))
              xnT_p = s1.enter_context(tc.tile_pool(name="xnT", bufs=1))
              ptA_ps = s1.enter_context(tc.tile_pool(name="ptA", bufs=1,
                                                     space="PSUM"))

              xts = x_p.tile([128, NT, D], BF16, name="xts")
              muvar = st_p.tile([128, NT, 2], F32, name="muvar")
              # x loads split across the sync and gpsimd DMA queues so
              # the tiles land twice as fast.
              for tt in range(NT):
                  eng = nc.sync if tt % 2 == 0 else nc.gpsimd
                  eng.dma_start(xts[:, tt, :],
                                x_nat[tt * 128:(tt + 1) * 128, :])
              mu_all = muvar[:, :, 0:1].rearrange("p a b -> p (a b)")
              var_all = muvar[:, :, 1:2].rearrange("p a b -> p (a b)")
              sd_all = st_p.tile([128, NT], F32, name="sd_all")
              rs_all = st_p.tile([128, NT], F32, name="rs_all")
              bias2 = st_p.tile([128, NT], F32, name="bias2")
              xnT = xnT_p.tile([128, KT, N], BF16, name="xnT")

              # batches of 4 tiles: stats -> batch sqrt/recip -> xn +
              # transpose + evac, so pass 2 of batch b overlaps pass 1
              # of batch b+1 instead of waiting for all 16 tiles.
              for b in range(NT // 4):
                  bs = slice(4 * b, 4 * b + 4)
                  for j in range(4):
                      tt = 4 * b + j
                      bn6 = scr_p.tile([128, 6], F32, tag="bn6")
                      nc.vector.bn_stats(bn6[:], xts[:, tt, :])
                      nc.vector.bn_aggr(muvar[:, tt, :], bn6[:])
                  nc.scalar.activation(sd_all[:, bs], var_all[:, bs],
                                       ACT.Sqrt, bias=eps_sb[:])
                  nc.vector.reciprocal(rs_all[:, bs], sd_all[:, bs])
                  nc.vector.scalar_tensor_tensor(
                      bias2[:, bs], mu_all[:, bs], -1.0, rs_all[:, bs],
                      op0=OP.mult, op1=OP.mult)
                  for j in range(4):
                      tt = 4 * b + j
                      xn = xn_p.tile([128, D], BF16, tag="xn")
                      if tt % 2 == 0:
                          nc.scalar.activation(
                              xn[:], xts[:, tt, :], ACT.Identity,
                              bias=bias2[:, tt:tt + 1],
                              scale=rs_all[:, tt:tt + 1])
                      else:
                          nc.vector.tensor_scalar(
                              xn[:], xts[:, tt, :], muvar[:, tt, 0:1],
                              rs_all[:, tt:tt + 1], op0=OP.subtract,
                              op1=OP.mult)
                      pt = ptA_ps.tile([128, KT, 128], BF16, tag="pt")
                      for ft in range(KT):
                          nc.tensor.transpose(
                              pt[:, ft, :], xn[:, ft * 128:(ft + 1) * 128],
                              ident_bf[:])
                      if tt % 2 == 0:
                          nc.vector.tensor_copy(
                              xnT[:, :, tt * 128:(tt + 1) * 128], pt[:])
                      else:
                          nc.scalar.copy(
                              xnT[:, :, tt * 128:(tt + 1) * 128], pt[:])

              # ---- Stage B: QKV + RoPE per pair ----
              with ExitStack() as s2:
                  pq_ps = s2.enter_context(tc.tile_pool(name="pq", bufs=2,
                                                        space="PSUM"))
                  pr_ps = s2.enter_context(tc.tile_pool(name="pr", bufs=2,
                                                        space="PSUM"))
                  ptV_ps = s2.enter_context(tc.tile_pool(name="ptV", bufs=1,
                                                         space="PSUM"))
                  t_p = s2.enter_context(tc.tile_pool(name="t", bufs=3))
                  t1_p = s2.enter_context(tc.tile_pool(name="t1", bufs=2))
                  vsb_p = s2.enter_context(tc.tile_pool(name="vsb", bufs=2))

                  def bm(m):
                      return beta_sb[:, m:m + 1]

                  def qkv_mm(psum_ap, m, half):
                      ms = slice(m * 128, (m + 1) * 128)
                      for nn in range(2):
                          cs = slice(half * 1024 + nn * 512,
                                     half * 1024 + (nn + 1) * 512)
                          for kt in range(KT):
                              nc.tensor.matmul(
                                  psum_ap[:, nn * 512:(nn + 1) * 512],
                                  wq_sb[:, kt, ms], xnT[:, kt, cs],
                                  start=(kt == 0), stop=(kt == KT - 1))

                  for p in range(2):
                      vsb = vsb_p.tile([128, N], BF16, tag=f"vsb{p}")
                      for half in range(2):
                          hs = slice(half * 1024, (half + 1) * 1024)
                          for sec, dst in ((0, ks[p]), (1, qs[p])):
                              m = 3 * p + sec
                              pq = pq_ps.tile([128, 1024], F32, tag="pq")
                              qkv_mm(pq, m, half)
                              # t = raw + beta (bf16), roll via perm matmul
                              t = t_p.tile([128, 1024], BF16, tag="t")
                              nc.scalar.add(t[:], pq[:], bm(m))
                              # dst = t*cos + roll(t)*sin; t*cos
                              # alternates GpSimd/DVE to balance load.
                              # roll psum is half-width double-buffered
                              # so consecutive units overlap.
                              t1 = t1_p.tile([128, 1024], BF16, tag="t1")
                              t1e = nc.gpsimd if (half + sec) % 2 else \
                                  nc.vector
                              t1e.tensor_tensor(
                                  t1[:], t[:], cos_sb[:, hs], op=OP.mult)
                              for nn in range(2):
                                  ns = slice(half * 1024 + nn * 512,
                                             half * 1024 + (nn + 1) * 512)
                                  prn = pr_ps.tile([128, 512], F32,
                                                   tag="pr", name="prn")
                                  nc.tensor.matmul(
                                      prn[:], r2_sb[:],
                                      t[:, nn * 512:(nn + 1) * 512],
                                      start=True, stop=True)
                                  nc.vector.scalar_tensor_tensor(
                                      dst[:, ns], prn[:], 0.0, sin_sb[:, ns],
                                      op0=OP.add, op1=OP.mult)
                              nc.vector.tensor_tensor(
                                  dst[:, hs], dst[:, hs], t1[:], op=OP.add)
                          # v
                          m = 3 * p + 2
                          pv = pq_ps.tile([128, 1024], F32, tag="pq")
                          qkv_mm(pv, m, half)
                          nc.vector.tensor_scalar_add(vsb[:, hs], pv[:], bm(m))
                      # transpose v to ktok-major, 4 tiles per psum bank,
                      # one fused strided evac per group into the paired
                      # [v_h0|1|v_h1|1] layout.
                      for g in range(NT // 4):
                          ptV = ptV_ps.tile([128, 4, 128], BF16, tag="ptV")
                          for j in range(4):
                              mt = 4 * g + j
                              nc.tensor.transpose(
                                  ptV[:, j, :],
                                  vsb[:, mt * 128:(mt + 1) * 128], ident_bf[:])
                          dstv = vap[p][:, 4 * g:4 * g + 4, :].rearrange(
                              "p m (h d) -> p m h d", h=2, d=65)[:, :, :, 0:64]
                          nc.scalar.copy(
                              dstv, ptV.rearrange("p m (h d) -> p m h d",
                                                  h=2, d=64))

          # ---- Stage C: attention per pair (+ interleaved out-proj) ----
          with ExitStack() as s3:
              s_ps = s3.enter_context(tc.tile_pool(name="sps", bufs=2,
                                                   space="PSUM"))
              a_ps = s3.enter_context(tc.tile_pool(name="aps", bufs=1,
                                                   space="PSUM"))
              po_ps = s3.enter_context(tc.tile_pool(name="pops", bufs=2,
                                                    space="PSUM"))
              p_pool = s3.enter_context(tc.tile_pool(name="pp", bufs=2))
              nrm_p = s3.enter_context(tc.tile_pool(name="nrm", bufs=2))
              ye_p = s3.enter_context(tc.tile_pool(name="ye", bufs=3))

              scale = float(DH) ** -0.5

              def boundary_pieces(pp, pqc, pacc, pP, tail=False):
                  """Previous-qchunk epilogue as (target_mt, fn) pieces.
                  PE work is chopped into <=~0.45us pieces scheduled at
                  the mt where their inputs are ready, so the in-order
                  PE stream never blocks on the slow DVE reciprocal
                  (four [1,256] chunks, ~1.7us each). tail=True is the
                  final drain: latency-optimized (D row first on DVE,
                  acc evac + ye evacs on the now-idle ACT, fp32 bcast
                  quarters gated per recip chunk)."""
                  qsl_n = slice(pqc * W, (pqc + 1) * W)
                  ou16 = nrm_p.tile([64, 2, W], BF16, tag="ou16")
                  # D row lives on one partition; 1/D at 5.7ns/elem on
                  # DVE would serialize ~6us. Instead: spread D across
                  # 128 partitions via PE transposes (column 0 of each
                  # [128,128] block), reciprocal on [128,4] (cheap),
                  # transpose back, and broadcast the bf16 result.
                  ds = nrm_p.tile([128, 2 * W], BF16, tag="ds")
                  rts = nrm_p.tile([128, 8], F32, tag="rts")
                  rts16 = nrm_p.tile([128, 136], BF16, tag="rts16")
                  rr16 = nrm_p.tile([1, 2 * W], BF16, tag="rr16")
                  hold = {}

                  def pv_flush():
                      for hh in range(2):
                          nc.tensor.matmul(
                              pacc[0:65, hh, :],
                              vap[pp][:, NT - 1, hh * 65:(hh + 1) * 65],
                              pP[:, NT - 1, hh, :], start=False, stop=True)
                      if tail:
                          nc.scalar.copy(ou16[:], pacc[0:64, :, :])
                      else:
                          nc.vector.tensor_copy(ou16[:], pacc[0:64, :, :])
                      nc.vector.tensor_copy(
                          ds[0:1, :], pacc[64:65, :, :].rearrange(
                              "p a b -> p (a b)"))

                  def tblk(g):      # transpose 4 D segments into psum
                      def f():
                          if g == 0:
                              hold["ptf"] = po_ps.tile([128, W], F32,
                                                       tag="po", name="ptf")
                          pt16 = hold["ptf"][:].bitcast(BF16)
                          for j in range(4):
                              k = 4 * g + j
                              nc.tensor.transpose(
                                  pt16[:, k * 128:(k + 1) * 128],
                                  ds[:, k * 128:(k + 1) * 128], ident_bf[:])
                          if g == 1:
                              cols = pt16.rearrange(
                                  "p (j c) -> p j c", j=8, c=128)[:, :, 0]
                              nc.vector.reciprocal(rts[:], cols)
                              nc.vector.tensor_copy(rts16[:, 0:8], rts[:])
                      return f

                  def tback(g):
                      # shifted transposes: input column-offset j puts
                      # 1/D segment j on psum ROW 0, evacuated into one
                      # [1, 2W] bf16 row for the partition-0 broadcast.
                      def f():
                          if g == 0:
                              hold["ptb"] = po_ps.tile([128, W], F32,
                                                       tag="po", name="ptb")
                          pt16 = hold["ptb"][:].bitcast(BF16)
                          for c in range(4):
                              j = 4 * g + c
                              nc.tensor.transpose(
                                  pt16[:, (4 * g + c) * 128:
                                       (4 * g + c + 1) * 128],
                                  rts16[:, j:j + 128], ident_bf[:])
                          if g == 1:
                              nc.vector.tensor_copy(rr16[:], pt16[0:1, :])
                      return f

                  def bcast(hh):
                      def f():
                          rbp = po_ps.tile([128, W], F32, tag="po",
                                           name=f"rbp{hh}")
                          hold[f"rbp{hh}"] = rbp
                          for c in range(4):
                              j = hh * 4 + c
                              nc.tensor.matmul(
                                  rbp[0:64, c * 128:(c + 1) * 128],
                                  ones_bf[:, :],
                                  rr16[0:1, j * 128:(j + 1) * 128],
                                  start=True, stop=True)
                      return f

                  def nmul():
                      for hh in range(2):
                          nc.vector.tensor_tensor(
                              outn[pp][hh * 64:hh * 64 + 64, qsl_n],
                              ou16[:, hh, :], hold[f"rbp{hh}"][0:64, :],
                              op=OP.mult)

                  def oproj(mi):
                      def f():
                          po = po_ps.tile([128, W], F32, tag="po",
                                          name=f"po{mi}")
                          for pr in range(2):
                              nc.tensor.matmul(
                                  po[:],
                                  wout_sb[:, pr, mi * 128:(mi + 1) * 128],
                                  outn[pr][:, qsl_n], start=(pr == 0),
                                  stop=(pr == 1))
                          ye = ye_p.tile([128, W], BF16, tag="ye")
                          if tail:
                              nc.scalar.copy(ye[:], po[:])
                          else:
                              nc.vector.tensor_copy(ye[:], po[:])
                          nc.sync.dma_start(
                              y[mi * 128:(mi + 1) * 128, qsl_n], ye[:])
                      return f

                  base = [(1, pv_flush), (3, tblk(0)), (4, tblk(1)),
                          (5, tback(0)), (6, tback(1)), (7, bcast(0)),
                          (8, bcast(1)), (9, nmul)]
                  if tail:
                      return base + [(10 + mi, oproj(mi)) for mi in range(4)]
                  if pp == 1:
                      base += [(10 + mi, oproj(mi)) for mi in range(4)]
                  return base

              pend = []
              for p in range(2):
                  for qc in range(N // W):
                      qsl = slice(qc * W, (qc + 1) * W)
                      P = p_pool.tile([128, NT, 2, W], BF16, name="P",
                                      tag="P")
                      acc = None
                      pv_done = 0    # PVs deferred past the acc-release
                      for mt in range(NT):
                          s3t = s_ps.tile([128, 2, W], F32, tag="s3")
                          for hh in range(2):
                              hsl = slice(hh * 64, hh * 64 + 64)
                              nc.tensor.matmul(
                                  s3t[:, hh, :],
                                  ks[p][hsl, mt * 128:(mt + 1) * 128],
                                  qs[p][hsl, qsl],
                                  start=True, stop=True)
                          nc.scalar.activation(
                              P[:, mt, :, :], s3t[:], ACT.Exp, scale=scale)
                          while pend and pend[0][0] <= mt:
                              pend.pop(0)[1]()
                          if mt == 1:
                              acc = a_ps.tile([128, 2, W], F32, name="acc",
                                              tag="acc")
                          # PV stream: none at mts 1-3 (the prev qchunk's
                          # acc evac hasn't freed the banks yet; a parked
                          # PV would block the in-order PE), then catch
                          # up two per mt.
                          if mt >= 4:
                              tgt = min(mt, NT - 1)
                              while pv_done < tgt:
                                  j = pv_done
                                  for hh in range(2):
                                      nc.tensor.matmul(
                                          acc[0:65, hh, :],
                                          vap[p][:, j,
                                                 hh * 65:(hh + 1) * 65],
                                          P[:, j, hh, :],
                                          start=(j == 0), stop=False)
                                  pv_done += 1
                                  if pv_done >= min(2 * (mt - 3), tgt):
                                      break
                      while pend:
                          pend.pop(0)[1]()
                      pend = boundary_pieces(p, qc, acc, P,
                                             tail=(p == 1 and
                                                   qc == N // W - 1))
              while pend:
                  pend.pop(0)[1]()

    _split_multiwait(nc)
    return nc


def _host_prep(x, ln_gamma, ln_beta, w_qkv, w_out):
    """Build the 8 per-core input maps."""
    import ml_dtypes
    f32 = np.float32
    bf16 = ml_dtypes.bfloat16
    pos = np.arange(N, dtype=f32)[:, None]
    idx = np.arange(DH, dtype=f32)[None, :]
    angle = pos / (f32(10000.0) ** (idx / f32(DH)))       # [N, DH]
    cos2 = np.ascontiguousarray(np.tile(np.cos(angle).T, (2, 1))).astype(bf16)
    sin2 = np.ascontiguousarray(np.tile(np.sin(angle).T, (2, 1))).astype(bf16)
    ident = np.eye(128, dtype=f32)
    roll64 = np.zeros((64, 64), f32)
    for p in range(64):
        roll64[(p - 1) % 64, p] = 1.0     # lhsT[src, dst]: dst p <- src p-1
    r2 = np.zeros((128, 128), f32)
    r2[0:64, 0:64] = roll64
    r2[64:128, 64:128] = roll64
    r2 = r2.astype(bf16)

    wg = (w_qkv * ln_gamma[:, None]).astype(f32)          # [512, 1536]
    beta_row = (ln_beta @ w_qkv).astype(f32)              # [1536]

    def head_block(a, sec, h):    # sec 0=q 1=k 2=v, global head h
        return a[..., sec * 512 + h * DH: sec * 512 + (h + 1) * DH]

    in_maps = []
    for c in range(8):
        bi, hg = c // 2, c % 2
        hs = [4 * hg + i for i in range(HPC)]
        mts, bcols = [], []
        # M-tile order: k01 q01 v01 k23 q23 v23
        for pr in range(2):
            for sec in (1, 0, 2):
                mts.append(np.concatenate(
                    [head_block(wg, sec, hs[2 * pr]),
                     head_block(wg, sec, hs[2 * pr + 1])], axis=1))
                bcols.append(np.concatenate(
                    [head_block(beta_row, sec, hs[2 * pr]),
                     head_block(beta_row, sec, hs[2 * pr + 1])]))
        wqkv_c = np.ascontiguousarray(
            np.concatenate(mts, axis=1)).astype(bf16)     # [512, 768]
        beta_c = np.stack(bcols, axis=1).astype(f32)      # [128, 6]
        wout_c = np.ascontiguousarray(
            w_out[hg * 256:(hg + 1) * 256, :]).astype(bf16)
        in_maps.append({
            "x_nat": np.ascontiguousarray(x[bi]).astype(bf16),
            "wqkv": wqkv_c,
            "beta_mt": beta_c,
            "r2": r2,
            "wout": wout_c,
            "cos2": cos2,
            "sin2": sin2,
            "ident": ident,
        })
    return in_maps


_NC = None


def kernel(x, ln_gamma, ln_beta, w_qkv, w_out, b_out, **run_kwargs):
    global _NC
    x = np.asarray(x, dtype=np.float32)
    assert x.shape == (B, N, D), x.shape
    if _NC is None:
        _NC = build_nc()
    in_maps = _host_prep(np.asarray(x), np.asarray(ln_gamma),
                         np.asarray(ln_beta), np.asarray(w_qkv),
                         np.asarray(w_out))
    res = run_bass_kernel_spmd(_NC, in_maps, core_ids=list(range(8)), **run_kwargs)
    out = np.empty((B, N, D), dtype=np.float32)
    for bi in range(B):
        part = (res.results[2 * bi]["y"].astype(np.float32)
                + res.results[2 * bi + 1]["y"].astype(np.float32))
        out[bi] = part.T + np.asarray(b_out, dtype=np.float32)
    kernel.last_results = res
    return out
